# revision 67
# baseline (speedup 1.0000x reference)
"""Trainium2 Bass kernel for causal self-attention with rotary + T5-style
relative-position bias (nn_CausalSelfAttention_27195732918417).

Sharding: 8 cores = 2 batches x 4 head-groups (4 heads each).
Each core computes its 4 heads end-to-end and a partial output projection;
the host sums the 4 partials per batch.

v3 design notes:
- softmax shift-invariance: the T5 bucket saturates at bucket 31 for all
  distances >= 113, so subtracting the per-head bucket-31 bias from the
  whole table leaves a bias of exactly 0 for all "far" blocks (block
  offset >= 256).  Far blocks need no bias multiply; the Toeplitz
  exp-bias table only needs 640 columns.
- fp8 (e4m3) DoubleRow matmuls: QKV projections for chunks 1-3 (weights
  host-scaled x32 to avoid fp8 subnormals, undone in the psum drain) and
  the attention AV for all full (non-diagonal) blocks.  V is padded to
  128 columns per head (64 v + ones col + 63 zeros) so the softmax
  denominator rides in the same DR matmul (stream-bound: free).
  Chunk 0 stays bf16: short-prefix rows average over few positions and
  need the precision.  exp() writes fp8 directly for far blocks.
- interleaved emission: the PE instruction stream alternates attention
  (ch), QKV (ch+1) and proj (ch-1) work so no engine starves.
- output in bf16 (host accumulates partials in f32).

Self-contained: hardcodes B=2, T=2048, C=1024, H=16, D=64.
"""

import math
import sys
import types
from collections import deque

import numpy as np
import ml_dtypes

# ---------------------------------------------------------------------------
# Environment patches (axon agent container)
# ---------------------------------------------------------------------------


def _install_ntff_hook():
    """Provide antenv.axon_hooks (missing in this image) so trace=True works."""
    try:
        from antenv.axon_hooks import get_axon_ntff_profile_hook  # noqa: F401
        return
    except ImportError:
        pass
    try:
        from trn_agent_boot.trn_boot import _ntff_profile_via_ctypes
        hook = _ntff_profile_via_ctypes('/opt/axon/libaxon_pjrt.so')
    except Exception:
        hook = None
    mod = types.ModuleType('antenv.axon_hooks')
    mod.get_axon_ntff_profile_hook = lambda: hook
    mod.set_axon_ntff_profile_hook = lambda h: None
    sys.modules['antenv.axon_hooks'] = mod


def _patch_tile_drain():
    """This container's walrus rejects >1 sync-wait per instruction.

    Two patches:
    1. tail drain: split its waits across multiple drain instructions
    2. general: before lowering, split any instruction with >1 waits by
       inserting standalone InstEventSemaphore carriers before it on the
       same engine (engine streams execute in order, so happens-before is
       preserved).
    """
    import concourse.mybir as mybir
    import concourse.tile as tile
    from concourse.tile import ScopedClock

    def _drain_and_barrier_split(self, tick_clock, wait_clock):
        nc = self.nc
        drain_inst = nc.sync.drain()
        wait_clock.add_sem_waits(
            drain_inst.ins, ScopedClock({None: tick_clock.global_clock})
        )
        si = drain_inst.ins.sync_info
        waits = list(si.on_wait) if si and si.on_wait else []
        if len(waits) > 1:
            si.on_wait = waits[:1]
            for w in waits[1:]:
                extra = nc.sync.drain()
                esi = extra.ins.sync_info
                if esi is None:
                    extra.ins.sync_info = mybir.SyncInfo(on_wait=[w], on_update=[])
                else:
                    esi.on_wait = [w]

        nc.all_engine_barrier()
        assert self.sems is not None
        popped = nc._tile_sem_poison_stack.pop()
        assert popped is self._sem_poison
        nc.clear_and_free_semaphores(list(self.sems.allocated().values()))
        nc.all_engine_barrier()

    tile.TileContext._drain_and_barrier = _drain_and_barrier_split

    orig_lower = tile.TileContext._lower_ordered_insts

    def _lower_split_waits(self, ordered):
        nc = self.nc
        for bb_name, insts in ordered.items():
            new_insts = []
            for inst in insts:
                si = getattr(inst, "sync_info", None)
                waits = list(si.on_wait) if si and si.on_wait else []
                if len(waits) > 1 and inst.engine != mybir.EngineType.Unassigned:
                    for w in waits[:-1]:
                        carrier = mybir.InstEventSemaphore(
                            name=nc.get_next_instruction_name(),
                            engine=inst.engine,
                            ins=[],
                            outs=[],
                            sync_info=mybir.SyncInfo(on_wait=[w], on_update=[]),
                        )
                        new_insts.append(carrier)
                    si.on_wait = waits[-1:]
                new_insts.append(inst)
            insts[:] = new_insts
        return orig_lower(self, ordered)

    tile.TileContext._lower_ordered_insts = _lower_split_waits


_install_ntff_hook()
_patch_tile_drain()

import concourse.bass as bass  # noqa: E402
import concourse.mybir as mybir  # noqa: E402
import concourse.tile as tile  # noqa: E402
from concourse.bass_utils import run_bass_kernel_spmd  # noqa: E402

# ---------------------------------------------------------------------------
# Problem constants
# ---------------------------------------------------------------------------
B, T, C = 2, 2048, 1024
H = 16            # total heads
D = 64            # head dim
HL = 4            # heads per core
DHL = HL * D      # 256 local channels
N_CORES = 8
NUM_BUCKETS = 32
MAX_DISTANCE = 128
ROTARY_BASE = 10000.0
SCALE = 1.0 / math.sqrt(D)
WS = 32.0         # fp8 weight pre-scale (avoids e4m3 subnormals)

F32 = mybir.dt.float32
BF16 = mybir.dt.bfloat16
FP8 = mybir.dt.float8e4
BF16_NP = ml_dtypes.bfloat16
FP8_NP = ml_dtypes.float8_e4m3
DR = mybir.MatmulPerfMode.DoubleRow

NT = T // 128     # 16 t-tiles
NKT = C // 128    # 8 contraction tiles
NCH = T // 512    # 4 streaming chunks
ETW = 640         # exp-bias table width (distances < 640 after b31 shift)


# ---------------------------------------------------------------------------
# Device program (identical on all cores; data differs)
# ---------------------------------------------------------------------------

def build_nc():
    from contextlib import ExitStack

    nc = bass.Bass()

    xTb = nc.dram_tensor("xTb", [128, NKT, 512], BF16, kind="ExternalInput")
    xT8 = nc.dram_tensor("xT8", [128, 3, NKT, 512], FP8, kind="ExternalInput")
    wq = nc.dram_tensor("wq", [128, NKT, DHL], BF16, kind="ExternalInput")
    wk = nc.dram_tensor("wk", [128, NKT, DHL], BF16, kind="ExternalInput")
    wv = nc.dram_tensor("wv", [128, NKT, HL * 65], BF16, kind="ExternalInput")
    wq8 = nc.dram_tensor("wq8", [128, NKT, DHL], FP8, kind="ExternalInput")
    wk8 = nc.dram_tensor("wk8", [128, NKT, DHL], FP8, kind="ExternalInput")
    wv8 = nc.dram_tensor("wv8", [128, NKT, HL * 65], FP8, kind="ExternalInput")
    wp = nc.dram_tensor("wp", [128, 2, C], BF16, kind="ExternalInput")
    cosT = nc.dram_tensor("cosT", [128, T], BF16, kind="ExternalInput")
    sinN = nc.dram_tensor("sinN", [128, T], BF16, kind="ExternalInput")
    # exp((bias - b31)/sqrt(D)) Toeplitz blocks, [pair][128][2 heads][ETW]
    etab = nc.dram_tensor("etab", [2, 128, 2, ETW], BF16, kind="ExternalInput")
    # signed rotate-half permutation matrix (out = perm.T @ in)
    perm = nc.dram_tensor("perm", [128, 128], BF16, kind="ExternalInput")
    out = nc.dram_tensor("out", [T, C], BF16, kind="ExternalOutput")

    with tile.TileContext(nc) as tc, ExitStack() as big:
        consts = big.enter_context(tc.tile_pool(name="consts", bufs=1))

        # const loads: ordered so the first chunk's deps arrive first;
        # split across the two DMA rings (gpsimd + sync issue ~1us each).
        warm = consts.tile([128, 512], BF16)
        nc.gpsimd.memset(warm, 0.0)
        # wq/wk ride the otherwise-idle scalar hwdge ring: low latency for
        # the startup-critical first fills, no contention with the sync
        # ring (softmax chains) or gpsimd ring (bulk consts)
        wq_sb = consts.tile([128, NKT, DHL], BF16)
        nc.scalar.dma_start(out=wq_sb, in_=wq[:])
        wk_sb = consts.tile([128, NKT, DHL], BF16)
        nc.scalar.dma_start(out=wk_sb, in_=wk[:])
        cos_sb = consts.tile([128, T], BF16)
        nc.gpsimd.dma_start(out=cos_sb, in_=cosT[:])
        sin_sb = consts.tile([128, T], BF16)
        nc.gpsimd.dma_start(out=sin_sb, in_=sinN[:])
        perm_sb = consts.tile([128, 128], BF16)
        nc.gpsimd.dma_start(out=perm_sb, in_=perm[:])
        wv_sb = consts.tile([128, NKT, HL * 65], BF16)
        nc.gpsimd.dma_start(out=wv_sb, in_=wv[:])
        wq8_sb = consts.tile([128, NKT, DHL], FP8)
        nc.gpsimd.dma_start(out=wq8_sb, in_=wq8[:])
        wk8_sb = consts.tile([128, NKT, DHL], FP8)
        nc.gpsimd.dma_start(out=wk8_sb, in_=wk8[:])
        wv8_sb = consts.tile([128, NKT, HL * 65], FP8)
        nc.gpsimd.dma_start(out=wv8_sb, in_=wv8[:])
        etab_sb = consts.tile([128, HL, ETW], BF16)
        for pr in range(2):
            nc.gpsimd.dma_start(out=etab_sb[:, 2 * pr:2 * pr + 2, :], in_=etab[pr])
        wp_sb = consts.tile([128, 2, C], BF16)
        nc.gpsimd.dma_start(out=wp_sb, in_=wp[:])

        # PE p-state warmup: a few garbage matmuls so the first real fills
        # run at full clock (PE needs ~3us of continuous work to ramp)
        warm_done = False

        # persistent activations
        acts = big.enter_context(tc.tile_pool(name="acts", bufs=1))
        qhat = acts.tile([128, 2, T], BF16)    # q^T rotary, heads (2m, 2m+1)
        khat = acts.tile([128, 2, T], BF16)
        vhat = acts.tile([128, NT, HL * 65], BF16)  # v natural + ones col/head
        vhat8 = acts.tile([128, NT, HL * 128], FP8)  # 128/head: v|ones|zeros
        ynhat = acts.tile([128, 2, T], BF16)   # normalized y^T for projection
        # zero-fill vhat8 once (cols 65:128 of each head stay zero forever)
        nc.gpsimd.memset(vhat8, 0.0)

        # working pools (whole-kernel scope)
        xpool = big.enter_context(tc.tile_pool(name="xpool", bufs=2))
        rot = big.enter_context(tc.tile_pool(name="rot", bufs=3))
        ppool = big.enter_context(tc.tile_pool(name="ppool", bufs=3))
        lpool = big.enter_context(tc.tile_pool(name="lpool", bufs=3))
        dpool = big.enter_context(tc.tile_pool(name="dpool", bufs=3, space="DRAM"))
        otpool = big.enter_context(tc.tile_pool(name="ot", bufs=3))

        qkps = big.enter_context(tc.tile_pool(name="qkps", bufs=2, space="PSUM"))
        spool = big.enter_context(tc.tile_pool(name="spool", bufs=2, space="PSUM"))
        ypool = big.enter_context(tc.tile_pool(name="ypool", bufs=1, space="PSUM"))

        # ------------------------------------------------------------------
        # QKV work units for one chunk (list of closures)
        # ------------------------------------------------------------------
        def qkv_units(ch):
            units = []
            fp8c = ch > 0
            sl = slice(ch * 512, (ch + 1) * 512)
            cell = {}
            xdt = FP8 if fp8c else BF16

            def load_x(ch=ch, xdt=xdt, fp8c=fp8c):
                xc = xpool.tile([128, NKT, 512], xdt, tag="xc", name="xc")
                cell['xc'] = xc
                if fp8c:
                    nc.sync.dma_start(out=xc, in_=xT8[:, ch - 1])
                else:
                    for q in range(4):
                        nc.sync.dma_start(
                            out=xc[:, 2 * q:2 * q + 2],
                            in_=xTb[:, 2 * q:2 * q + 2])
            units.append(load_x)

            def drain_qk(ps, qk2, half, fp8c):
                # psum -> joint bf16 tile [128, 2(qk), 512]
                if fp8c:
                    nc.vector.tensor_scalar_mul(qk2[:, half, :], ps, 1.0 / WS)
                else:
                    nc.vector.tensor_copy(qk2[:, half, :], ps)

            def rot_tail(qk2, m, sl):
                # signed rotate-half. ch0/ch1 (critical path to the next
                # attention start): PE permutation matmul, low latency.
                # ch2-3: SBUF-SBUF shift DMAs, latency hidden by the longer
                # preceding attention chunks.
                if ch <= 1:
                    qs2p = [qkps.tile([128, 512], F32, tag="qkps",
                                      name="qs2p") for _ in range(2)]
                    for half in range(2):
                        nc.tensor.matmul(
                            qs2p[half], lhsT=perm_sb, rhs=qk2[:, half, :],
                            start=True, stop=True)
                    shalves = qs2p
                else:
                    qs2 = rot.tile([128, 2, 512], BF16, tag="qs", name="qs")
                    for b0 in (0, 64):
                        nc.sync.dma_start(
                            out=qs2[b0:b0 + 32], in_=qk2[b0 + 32:b0 + 64])
                        nc.sync.dma_start(
                            out=qs2[b0 + 32:b0 + 64], in_=qk2[b0:b0 + 32])
                    shalves = [qs2[:, 0, :], qs2[:, 1, :]]
                for half, dst in ((0, qhat), (1, khat)):
                    u = rot.tile([128, 512], BF16, tag="u", name="u")
                    nc.vector.tensor_mul(u, qk2[:, half, :], cos_sb[:, sl])
                    t_t = rot.tile([128, 512], BF16, tag="t", name="t")
                    nc.vector.tensor_mul(t_t, shalves[half], sin_sb[:, sl])
                    nc.vector.tensor_add(dst[:, m, sl], u, t_t)

            def emit_m(m):
                msl = slice(m * 128, (m + 1) * 128)
                qk2_cell = {}
                mu = []

                def mk_qk2(qk2_cell=qk2_cell):
                    qk2_cell['t'] = rot.tile([128, 2, 512], BF16, tag="qk2",
                                             name="qk2")

                for wi, (wsb, wsb8) in enumerate(
                        ((wq_sb, wq8_sb), (wk_sb, wk8_sb))):
                    if fp8c:
                        def fill_h1(wsb8=wsb8, msl=msl, wi=wi,
                                    qk2_cell=qk2_cell, mk=mk_qk2):
                            if wi == 0:
                                mk()
                            ps = qkps.tile([128, 512], F32, tag="qkps",
                                           name="ps")
                            cell[('ps', wi)] = ps
                            xc = cell['xc']
                            for j in range(2):
                                nc.tensor.matmul(
                                    ps, lhsT=wsb8[:, 2 * j:2 * j + 2, msl],
                                    rhs=xc[:, 2 * j:2 * j + 2, :],
                                    start=(j == 0), stop=False, perf_mode=DR)

                        def fill_h2(wsb8=wsb8, msl=msl, wi=wi,
                                    qk2_cell=qk2_cell):
                            ps = cell[('ps', wi)]
                            xc = cell['xc']
                            for j in range(2, 4):
                                nc.tensor.matmul(
                                    ps, lhsT=wsb8[:, 2 * j:2 * j + 2, msl],
                                    rhs=xc[:, 2 * j:2 * j + 2, :],
                                    start=False, stop=(j == 3), perf_mode=DR)
                            drain_qk(ps, qk2_cell['t'], wi, True)
                        mu.append(fill_h1)
                        mu.append(fill_h2)
                    else:
                        def fill_h1(wsb=wsb, msl=msl, wi=wi,
                                    qk2_cell=qk2_cell, mk=mk_qk2):
                            if wi == 0:
                                mk()
                            ps = qkps.tile([128, 512], F32, tag="qkps",
                                           name="ps")
                            cell[('ps', wi)] = ps
                            xc = cell['xc']
                            for kt in range(4):
                                nc.tensor.matmul(
                                    ps, lhsT=wsb[:, kt, msl], rhs=xc[:, kt, :],
                                    start=(kt == 0), stop=False)

                        def fill_h2(wsb=wsb, msl=msl, wi=wi,
                                    qk2_cell=qk2_cell):
                            ps = cell[('ps', wi)]
                            xc = cell['xc']
                            for kt in range(4, NKT):
                                nc.tensor.matmul(
                                    ps, lhsT=wsb[:, kt, msl], rhs=xc[:, kt, :],
                                    start=False, stop=(kt == NKT - 1))
                            drain_qk(ps, qk2_cell['t'], wi, False)
                        mu.append(fill_h1)
                        mu.append(fill_h2)

                def rotu(m=m, sl=sl, qk2_cell=qk2_cell):
                    rot_tail(qk2_cell['t'], m, sl)
                mu.append(rotu)
                return mu

            def vfill_units():
                vu = []
                for ts in range(4):
                    tt = ch * 4 + ts

                    def vfill(ts=ts, tt=tt, fp8c=fp8c):
                        xc = cell['xc']
                        tsl = slice(ts * 128, (ts + 1) * 128)
                        vp = qkps.tile([128, HL * 65], F32, tag="qkps",
                                       name="vp")
                        if fp8c:
                            for j in range(4):
                                nc.tensor.matmul(
                                    vp, lhsT=xc[:, 2 * j:2 * j + 2, tsl],
                                    rhs=wv8_sb[:, 2 * j:2 * j + 2, :],
                                    start=(j == 0), stop=(j == 3),
                                    perf_mode=DR)
                            nc.vector.tensor_scalar_mul(
                                vhat[:, tt, :], vp, 1.0 / WS)
                        else:
                            for kt in range(NKT):
                                nc.tensor.matmul(
                                    vp, lhsT=xc[:, kt, tsl],
                                    rhs=wv_sb[:, kt, :],
                                    start=(kt == 0), stop=(kt == NKT - 1))
                            nc.vector.tensor_copy(vhat[:, tt, :], vp)
                        for h in range(HL):
                            nc.gpsimd.memset(
                                vhat[:, tt, 65 * h + 64:65 * h + 65], 1.0)
                        # fp8 copy (v + ones cols; zero cols untouched)
                        src = vhat[:, tt, :].rearrange("p (h c) -> p h c", h=HL)
                        dst8 = vhat8[:, tt, :].rearrange(
                            "p (h c) -> p h c", h=HL)[:, :, 0:65]
                        nc.gpsimd.tensor_copy(dst8, src)
                    vu.append(vfill)
                return vu

            # order: x, m0 q/k/rot, v fills, m1 q/k/rot -- so the next
            # chunk's pair-0 attention can start as early as possible
            units.extend(emit_m(0))
            units.extend(vfill_units())
            units.extend(emit_m(1))
            return units

        # ------------------------------------------------------------------
        # proj work units for one chunk
        # ------------------------------------------------------------------
        def proj_units(chp):
            units = []
            for tt in range(4 * chp, 4 * chp + 4):
                def punit(tt=tt):
                    tsl = slice(tt * 128, (tt + 1) * 128)
                    pp = spool.tile([128, 2, 512], F32, tag="s", name="pp")
                    for h2 in range(2):
                        nsl = slice(h2 * 512, (h2 + 1) * 512)
                        for kt in range(2):
                            nc.tensor.matmul(
                                pp[:, h2, :],
                                lhsT=ynhat[:, kt, tsl],
                                rhs=wp_sb[:, kt, nsl],
                                start=(kt == 0), stop=(kt == 1))
                    ot = otpool.tile([128, C], BF16, tag="ot", name="ot")
                    nc.vector.tensor_copy(ot[:, 0:512], pp[:, 0, :])
                    nc.scalar.copy(ot[:, 512:1024], pp[:, 1, :])
                    nc.gpsimd.dma_start(out=out[tsl, :], in_=ot)
                units.append(punit)
            return units

        # ------------------------------------------------------------------
        # attention for one chunk, interleaving pending units
        # ------------------------------------------------------------------
        def emit_attention(ch, pending, reserve=3):
            ic = ch
            i0, i1 = 512 * ic, 512 * (ic + 1)
            jt_hi = min(NT, 4 * (ic + 1))
            full = 4 * ic                      # full blocks; always even
            total_iters = 2 * (full + 4)  # pop points: every jt, both pairs
            n_pend = len(pending)
            reserve = min(3, n_pend)  # keep some PE work for the tail chain
            n_paced = n_pend - reserve
            it = 0
            popped = 0

            def pop_quota():
                nonlocal popped, it
                it += 1
                want = min(n_paced,
                           (n_paced * it + total_iters - 1) // total_iters)
                while popped < want:
                    pending.popleft()()
                    popped += 1

            for pair in range(2):
                ys = [ypool.tile([128, 512], F32, tag=f"y{a}", name=f"y{a}")
                      for a in range(2)]
                ysb = lpool.tile([128, 2, 512], F32, tag="ysb", name="ysb")
                # full blocks: fp8 DoubleRow AV over jt pairs, pipelined
                # at single-jt granularity (sp ring keeps 1-jt lookahead)
                pe8 = None
                for jt in range(full):
                    ji = jt % 2
                    sp = spool.tile([128, 2, 512], F32, tag="s", name="sp")
                    for a in range(2):
                        asl = slice(64 * a, 64 * a + 64)
                        nc.tensor.matmul(
                            sp[:, a, :],
                            lhsT=khat[asl, pair, jt * 128:(jt + 1) * 128],
                            rhs=qhat[asl, pair, i0:i1],
                            start=True, stop=True)
                    if ji == 0:
                        pe8 = ppool.tile([128, 2, 2, 512], FP8, tag="pe8",
                                         name="pe8")
                    if jt == full - 1:  # jt=4ic-1: near block, needs bias
                        pt = ppool.tile([128, 2, 512], BF16, tag="pt",
                                        name="pt")
                        nc.scalar.activation(
                            pt, sp, mybir.ActivationFunctionType.Exp,
                            scale=SCALE)
                        nc.vector.tensor_mul(
                            pe8[:, 1], pt,
                            etab_sb[:, 2 * pair:2 * pair + 2, 128:640])
                    else:
                        nc.scalar.activation(
                            pe8[:, ji], sp,
                            mybir.ActivationFunctionType.Exp, scale=SCALE)
                    if ji == 1:
                        jt0 = jt - 1
                        for a in range(2):
                            h = 2 * pair + a
                            nc.tensor.matmul(
                                ys[a],
                                lhsT=vhat8[:, jt0:jt0 + 2,
                                           128 * h:128 * h + 128],
                                rhs=pe8[:, :, a, :],
                                start=(jt0 == 0), stop=False, perf_mode=DR,
                                skip_group_check=True)
                    pop_quota()
                # diagonal blocks (bf16)
                for jt in range(4 * ic, jt_hi):
                    i_lo = jt * 128
                    n = i1 - i_lo
                    sp = spool.tile([128, 2, 512], F32, tag="s", name="sp")
                    for a in range(2):
                        asl = slice(64 * a, 64 * a + 64)
                        nc.tensor.matmul(
                            sp[:, a, :n],
                            lhsT=khat[asl, pair, jt * 128:(jt + 1) * 128],
                            rhs=qhat[asl, pair, i_lo:i1],
                            start=True, stop=True)
                    pt = ppool.tile([128, 2, 512], BF16, tag="pt", name="pt")
                    nc.scalar.activation(
                        pt[:, :, :n], sp[:, :, :n],
                        mybir.ActivationFunctionType.Exp, scale=SCALE)
                    pe = ppool.tile([128, 2, 512], BF16, tag="pe", name="pe")
                    nc.vector.tensor_mul(
                        pe[:, :, :n], pt[:, :, :n],
                        etab_sb[:, 2 * pair:2 * pair + 2, 0:n])
                    for a in range(2):
                        h = 2 * pair + a
                        nc.tensor.matmul(
                            ys[a][0:65, i_lo - i0:512],
                            lhsT=vhat[:, jt, 65 * h:65 * h + 65],
                            rhs=pe[:, a, :n],
                            start=(ic == 0 and jt == 0),
                            stop=(jt == jt_hi - 1),
                            skip_group_check=True)
                    pop_quota()

                # eagerly drain y psum -> sbuf so the next pair's AV can
                # reuse the psum banks without waiting for the chain below
                nc.vector.tensor_copy(ysb[0:65, 0, :], ys[0][0:65, :])
                nc.vector.tensor_copy(ysb[0:65, 1, :], ys[1][0:65, :])
                # softmax denominators + normalize (batched a=0,1)
                ld = dpool.tile([1, 1024], F32, tag="ld", name="ld")
                nc.sync.dma_start(out=ld, in_=ysb[64:65].rearrange("p a c -> p (a c)"))
                l128 = lpool.tile([128, 8], F32, tag="l128", name="l128")
                nc.sync.dma_start(
                    out=l128, in_=ld.rearrange("a (p c) -> (a p) c", p=128))
                r128 = lpool.tile([128, 8], F32, tag="r128", name="r128")
                nc.vector.reciprocal(r128, l128)
                rd = dpool.tile([1, 1024], F32, tag="rd", name="rd")
                nc.sync.dma_start(
                    out=rd.rearrange("a (p c) -> (a p) c", p=128), in_=r128)
                rb2 = lpool.tile([64, 2, 512], F32, tag="rb2", name="rb2")
                r_bcast = bass.AP(
                    tensor=rd.tensor, offset=rd.offset,
                    ap=[[0, 64]] + list(rd.rearrange(
                        "a (h c) -> a h c", h=2).ap[1:]))
                nc.sync.dma_start(out=rb2, in_=r_bcast)
                for a in range(2):
                    nc.vector.tensor_mul(
                        ynhat[64 * a:64 * a + 64, pair, i0:i1],
                        ysb[0:64, a, :], rb2[:, a, :])

            # drain any leftover units
            while pending:
                pending.popleft()()

        # ------------------------------------------------------------------
        # main schedule
        # ------------------------------------------------------------------
        # PE warmup: garbage matmuls on the memset tile ramp the clock
        # while the const DMAs stream in; also prime the exp act table
        wps = spool.tile([128, 2, 512], F32, tag="s", name="wps")
        for i in range(6):
            nc.tensor.matmul(wps[:, 0, :], lhsT=warm[:, 0:128], rhs=warm,
                             start=(i == 0), stop=(i == 5))
        warmx = consts.tile([1, 8], BF16)
        nc.scalar.activation(warmx, warm[0:1, 0:8],
                             mybir.ActivationFunctionType.Exp)

        u0 = qkv_units(0)
        # emit x + m0 + v fills now; m1 units become att(0) filler
        for u in u0[:10]:
            u()
        pend0 = deque(u0[10:])
        pend0.extend(qkv_units(1))
        emit_attention(0, pend0)
        pend1 = deque(qkv_units(2))
        pend1.extend(proj_units(0))
        emit_attention(1, pend1)
        emit_attention(2, deque(qkv_units(3)))
        pend3 = deque(proj_units(1))
        pend3.extend(proj_units(2))
        emit_attention(3, pend3, reserve=3)
        for u in proj_units(NCH - 1):
            u()

    return nc


# ---------------------------------------------------------------------------
# Host-side input preparation
# ---------------------------------------------------------------------------

def _rotary_tables():
    inv_freq = (1.0 / (ROTARY_BASE ** (
        np.arange(0, D, 2, dtype=np.float32) / D))).astype(np.float32)
    t = np.arange(T, dtype=np.float32)
    freqs = np.einsum('i,j->ij', t, inv_freq).astype(np.float32)  # [T, 32]
    freqs = np.concatenate([freqs, freqs], axis=1)                # [T, 64]
    cos = np.cos(freqs).T.astype(np.float32)                      # [64, T]
    sin = np.sin(freqs).T.astype(np.float32)
    # stack for two heads per 128-partition tile
    cosT = np.concatenate([cos, cos], axis=0)                     # [128, T]
    sinN = np.concatenate([sin, sin], axis=0).copy()
    # shifted-term coefficient indexed by DEST row (the shifted copy is
    # materialized before the multiply): rows 0:32 get -sin, 32:64 get +sin
    sinN[0:32] *= -1.0
    sinN[64:96] *= -1.0
    return (np.ascontiguousarray(cosT).astype(BF16_NP),
            np.ascontiguousarray(sinN).astype(BF16_NP))


def _perm_matrix():
    """Plain rotate-half permutation (signs live in sinN): swap 32-row
    halves within each 64-row head block."""
    P = np.zeros((128, 128), dtype=np.float32)
    for b in (0, 64):
        for j in range(32):
            P[b + 32 + j, b + j] = 1.0
            P[b + j, b + 32 + j] = 1.0
    return P.astype(BF16_NP)


def _bucket(d):
    """T5 causal relative-position bucket for distance d = i - j >= 0."""
    d = np.asarray(d)
    max_exact = NUM_BUCKETS // 2
    is_small = d < max_exact
    dsafe = np.maximum(d, 1).astype(np.float32)
    val = max_exact + (
        np.log(dsafe / max_exact) / math.log(MAX_DISTANCE / max_exact)
        * (NUM_BUCKETS - max_exact)
    ).astype(np.int32)
    val = np.minimum(val, NUM_BUCKETS - 1)
    return np.where(is_small, d, val)


def _etab_for_heads(rel_bias_table, heads):
    """exp((bias-b31)/sqrt(D)) block-Toeplitz table [len(heads), 128, ETW].
    Column k*128+ii, row jj -> distance 128k + ii - jj; negative -> 0 (mask).
    The per-head bucket-31 bias is subtracted (softmax shift-invariance), so
    any block at distance offset >= 256 is exactly 1.0 and skips the lookup.
    """
    ii = np.arange(128)
    jj = np.arange(128)
    out = np.zeros((len(heads), 128, ETW), dtype=np.float32)
    dmax = ETW
    dist_all = np.arange(0, dmax)
    buck = _bucket(dist_all)  # [ETW]
    g = {}
    for hi, h in enumerate(heads):
        b31 = rel_bias_table[NUM_BUCKETS - 1, h].astype(np.float32)
        gh = np.exp((rel_bias_table[buck, h].astype(np.float32) - b31) * SCALE)
        g[h] = gh
    for k in range(ETW // 128):
        dmat = 128 * k + ii[None, :] - jj[:, None]  # [jj, ii]
        valid = dmat >= 0
        dcl = np.clip(dmat, 0, dmax - 1)
        for hi, h in enumerate(heads):
            blk = np.where(valid, g[h][dcl], 0.0)
            out[hi, :, 128 * k:128 * (k + 1)] = blk
    return out.astype(BF16_NP)


_NC_CACHE = None


def _prearr(w):
    """[K, N] -> [128, K//128, N] partition-contiguous layout."""
    k, n = w.shape
    return np.ascontiguousarray(w.reshape(k // 128, 128, n).transpose(1, 0, 2))


def _pad_wv(wv_slice):
    """[C, 256] -> [C, 260]: per head 64 cols + a zero col (ones col target)."""
    out = np.zeros((C, HL * 65), dtype=np.float32)
    for h in range(HL):
        out[:, 65 * h:65 * h + 64] = wv_slice[:, 64 * h:64 * h + 64]
    return out


def _build_in_maps(inputs):
    x = np.asarray(inputs["x"], dtype=np.float32)
    Wq = np.asarray(inputs["Wq"], dtype=np.float32)
    Wk = np.asarray(inputs["Wk"], dtype=np.float32)
    Wv = np.asarray(inputs["Wv"], dtype=np.float32)
    Wp = np.asarray(inputs["Wp"], dtype=np.float32)
    rel_bias_table = np.asarray(inputs["rel_bias_table"], dtype=np.float32)

    cosT, sinN = _rotary_tables()
    in_maps = []
    for core in range(N_CORES):
        b = core // 4
        hg = core % 4
        heads = list(range(4 * hg, 4 * hg + 4))
        csl = slice(DHL * hg, DHL * (hg + 1))
        xt = x[b].T.astype(np.float32)           # [C, T]
        xr = np.ascontiguousarray(
            xt.reshape(NKT, 128, NCH, 512).transpose(1, 2, 0, 3))
        wvp = _pad_wv(Wv[:, csl])
        in_maps.append({
            "xTb": xr[:, 0].astype(BF16_NP),
            "xT8": xr[:, 1:].astype(FP8_NP),
            "wq": _prearr(Wq[:, csl]).astype(BF16_NP),
            "wk": _prearr(Wk[:, csl]).astype(BF16_NP),
            "wv": _prearr(wvp).astype(BF16_NP),
            "wq8": _prearr(Wq[:, csl] * WS).astype(FP8_NP),
            "wk8": _prearr(Wk[:, csl] * WS).astype(FP8_NP),
            "wv8": _prearr(wvp * WS).astype(FP8_NP),
            "wp": _prearr(Wp[csl, :]).astype(BF16_NP),
            "cosT": cosT,
            "sinN": sinN,
            "perm": _perm_matrix(),
            "etab": _etab_for_heads(rel_bias_table, heads).reshape(
                2, 2, 128, ETW).transpose(0, 2, 1, 3).copy(),
        })
    return in_maps


def kernel(x, Wq, bq, Wk, bk, Wv, bv, Wp, bp, rel_bias_table):
    global _NC_CACHE
    if _NC_CACHE is None:
        _NC_CACHE = build_nc()
    nc = _NC_CACHE

    in_maps = _build_in_maps({
        "x": x, "Wq": Wq, "Wk": Wk, "Wv": Wv, "Wp": Wp,
        "rel_bias_table": rel_bias_table,
    })

    res = run_bass_kernel_spmd(nc, in_maps, list(range(N_CORES)))

    out = np.zeros((B, T, C), dtype=np.float32)
    for core in range(N_CORES):
        out[core // 4] += np.asarray(res.results[core]["out"], dtype=np.float32)
    out += np.asarray(bp, dtype=np.float32)[None, None, :]
    return out


# revision 68
# speedup vs baseline: 1.0136x; 1.0136x over previous
"""Trainium2 Bass kernel for causal self-attention with rotary + T5-style
relative-position bias (nn_CausalSelfAttention_27195732918417).

Sharding: 8 cores = 2 batches x 4 head-groups (4 heads each).
Each core computes its 4 heads end-to-end and a partial output projection;
the host sums the 4 partials per batch.

v3 design notes:
- softmax shift-invariance: the T5 bucket saturates at bucket 31 for all
  distances >= 113, so subtracting the per-head bucket-31 bias from the
  whole table leaves a bias of exactly 0 for all "far" blocks (block
  offset >= 256).  Far blocks need no bias multiply; the Toeplitz
  exp-bias table only needs 640 columns.
- fp8 (e4m3) DoubleRow matmuls: QKV projections for chunks 1-3 (weights
  host-scaled x32 to avoid fp8 subnormals, undone in the psum drain) and
  the attention AV for all full (non-diagonal) blocks.  V is padded to
  128 columns per head (64 v + ones col + 63 zeros) so the softmax
  denominator rides in the same DR matmul (stream-bound: free).
  Chunk 0 stays bf16: short-prefix rows average over few positions and
  need the precision.  exp() writes fp8 directly for far blocks.
- interleaved emission: the PE instruction stream alternates attention
  (ch), QKV (ch+1) and proj (ch-1) work so no engine starves.
- output in bf16 (host accumulates partials in f32).

Self-contained: hardcodes B=2, T=2048, C=1024, H=16, D=64.
"""

import math
import sys
import types
from collections import deque

import numpy as np
import ml_dtypes

# ---------------------------------------------------------------------------
# Environment patches (axon agent container)
# ---------------------------------------------------------------------------


def _install_ntff_hook():
    """Provide antenv.axon_hooks (missing in this image) so trace=True works."""
    try:
        from antenv.axon_hooks import get_axon_ntff_profile_hook  # noqa: F401
        return
    except ImportError:
        pass
    try:
        from trn_agent_boot.trn_boot import _ntff_profile_via_ctypes
        hook = _ntff_profile_via_ctypes('/opt/axon/libaxon_pjrt.so')
    except Exception:
        hook = None
    mod = types.ModuleType('antenv.axon_hooks')
    mod.get_axon_ntff_profile_hook = lambda: hook
    mod.set_axon_ntff_profile_hook = lambda h: None
    sys.modules['antenv.axon_hooks'] = mod


def _patch_tile_drain():
    """This container's walrus rejects >1 sync-wait per instruction.

    Two patches:
    1. tail drain: split its waits across multiple drain instructions
    2. general: before lowering, split any instruction with >1 waits by
       inserting standalone InstEventSemaphore carriers before it on the
       same engine (engine streams execute in order, so happens-before is
       preserved).
    """
    import concourse.mybir as mybir
    import concourse.tile as tile
    from concourse.tile import ScopedClock

    def _drain_and_barrier_split(self, tick_clock, wait_clock):
        nc = self.nc
        drain_inst = nc.sync.drain()
        wait_clock.add_sem_waits(
            drain_inst.ins, ScopedClock({None: tick_clock.global_clock})
        )
        si = drain_inst.ins.sync_info
        waits = list(si.on_wait) if si and si.on_wait else []
        if len(waits) > 1:
            si.on_wait = waits[:1]
            for w in waits[1:]:
                extra = nc.sync.drain()
                esi = extra.ins.sync_info
                if esi is None:
                    extra.ins.sync_info = mybir.SyncInfo(on_wait=[w], on_update=[])
                else:
                    esi.on_wait = [w]

        nc.all_engine_barrier()
        assert self.sems is not None
        popped = nc._tile_sem_poison_stack.pop()
        assert popped is self._sem_poison
        nc.clear_and_free_semaphores(list(self.sems.allocated().values()))
        nc.all_engine_barrier()

    tile.TileContext._drain_and_barrier = _drain_and_barrier_split

    orig_lower = tile.TileContext._lower_ordered_insts

    def _lower_split_waits(self, ordered):
        nc = self.nc
        for bb_name, insts in ordered.items():
            new_insts = []
            for inst in insts:
                si = getattr(inst, "sync_info", None)
                waits = list(si.on_wait) if si and si.on_wait else []
                if len(waits) > 1 and inst.engine != mybir.EngineType.Unassigned:
                    for w in waits[:-1]:
                        carrier = mybir.InstEventSemaphore(
                            name=nc.get_next_instruction_name(),
                            engine=inst.engine,
                            ins=[],
                            outs=[],
                            sync_info=mybir.SyncInfo(on_wait=[w], on_update=[]),
                        )
                        new_insts.append(carrier)
                    si.on_wait = waits[-1:]
                new_insts.append(inst)
            insts[:] = new_insts
        return orig_lower(self, ordered)

    tile.TileContext._lower_ordered_insts = _lower_split_waits


_install_ntff_hook()
_patch_tile_drain()

import concourse.bass as bass  # noqa: E402
import concourse.mybir as mybir  # noqa: E402
import concourse.tile as tile  # noqa: E402
from concourse.bass_utils import run_bass_kernel_spmd  # noqa: E402

# ---------------------------------------------------------------------------
# Problem constants
# ---------------------------------------------------------------------------
B, T, C = 2, 2048, 1024
H = 16            # total heads
D = 64            # head dim
HL = 4            # heads per core
DHL = HL * D      # 256 local channels
N_CORES = 8
NUM_BUCKETS = 32
MAX_DISTANCE = 128
ROTARY_BASE = 10000.0
SCALE = 1.0 / math.sqrt(D)
WS = 32.0         # fp8 weight pre-scale (avoids e4m3 subnormals)

F32 = mybir.dt.float32
BF16 = mybir.dt.bfloat16
FP8 = mybir.dt.float8e4
BF16_NP = ml_dtypes.bfloat16
FP8_NP = ml_dtypes.float8_e4m3
DR = mybir.MatmulPerfMode.DoubleRow

NT = T // 128     # 16 t-tiles
NKT = C // 128    # 8 contraction tiles
NCH = T // 512    # 4 streaming chunks
ETW = 640         # exp-bias table width (distances < 640 after b31 shift)


# ---------------------------------------------------------------------------
# Device program (identical on all cores; data differs)
# ---------------------------------------------------------------------------

def build_nc():
    from contextlib import ExitStack

    nc = bass.Bass()

    xTb = nc.dram_tensor("xTb", [128, NKT, 512], BF16, kind="ExternalInput")
    xT8 = nc.dram_tensor("xT8", [128, 3, NKT, 512], FP8, kind="ExternalInput")
    wq = nc.dram_tensor("wq", [128, NKT, DHL], BF16, kind="ExternalInput")
    wk = nc.dram_tensor("wk", [128, NKT, DHL], BF16, kind="ExternalInput")
    wv = nc.dram_tensor("wv", [128, NKT, HL * 65], BF16, kind="ExternalInput")
    wq8 = nc.dram_tensor("wq8", [128, NKT, DHL], FP8, kind="ExternalInput")
    wk8 = nc.dram_tensor("wk8", [128, NKT, DHL], FP8, kind="ExternalInput")
    wv8 = nc.dram_tensor("wv8", [128, NKT, HL * 65], FP8, kind="ExternalInput")
    wp = nc.dram_tensor("wp", [128, 2, C], BF16, kind="ExternalInput")
    cosT = nc.dram_tensor("cosT", [128, T], BF16, kind="ExternalInput")
    sinN = nc.dram_tensor("sinN", [128, T], BF16, kind="ExternalInput")
    # exp((bias - b31)/sqrt(D)) Toeplitz blocks, [pair][128][2 heads][ETW]
    etab = nc.dram_tensor("etab", [2, 128, 2, ETW], BF16, kind="ExternalInput")
    # signed rotate-half permutation matrix (out = perm.T @ in)
    perm = nc.dram_tensor("perm", [128, 128], BF16, kind="ExternalInput")
    out = nc.dram_tensor("out", [T, C], BF16, kind="ExternalOutput")

    with tile.TileContext(nc) as tc, ExitStack() as big:
        consts = big.enter_context(tc.tile_pool(name="consts", bufs=1))

        # const loads: ordered so the first chunk's deps arrive first;
        # split across the two DMA rings (gpsimd + sync issue ~1us each).
        warm = consts.tile([128, 512], BF16)
        nc.gpsimd.memset(warm, 0.0)
        wq_sb = consts.tile([128, NKT, DHL], BF16)
        nc.gpsimd.dma_start(out=wq_sb, in_=wq[:])
        wk_sb = consts.tile([128, NKT, DHL], BF16)
        nc.gpsimd.dma_start(out=wk_sb, in_=wk[:])
        cos_sb = consts.tile([128, T], BF16)
        nc.gpsimd.dma_start(out=cos_sb, in_=cosT[:])
        sin_sb = consts.tile([128, T], BF16)
        nc.gpsimd.dma_start(out=sin_sb, in_=sinN[:])
        perm_sb = consts.tile([128, 128], BF16)
        nc.gpsimd.dma_start(out=perm_sb, in_=perm[:])
        wv_sb = consts.tile([128, NKT, HL * 65], BF16)
        nc.gpsimd.dma_start(out=wv_sb, in_=wv[:])
        wq8_sb = consts.tile([128, NKT, DHL], FP8)
        nc.gpsimd.dma_start(out=wq8_sb, in_=wq8[:])
        wk8_sb = consts.tile([128, NKT, DHL], FP8)
        nc.gpsimd.dma_start(out=wk8_sb, in_=wk8[:])
        wv8_sb = consts.tile([128, NKT, HL * 65], FP8)
        nc.gpsimd.dma_start(out=wv8_sb, in_=wv8[:])
        etab_sb = consts.tile([128, HL, ETW], BF16)
        for pr in range(2):
            nc.gpsimd.dma_start(out=etab_sb[:, 2 * pr:2 * pr + 2, :], in_=etab[pr])
        wp_sb = consts.tile([128, 2, C], BF16)
        nc.gpsimd.dma_start(out=wp_sb, in_=wp[:])

        # PE p-state warmup: a few garbage matmuls so the first real fills
        # run at full clock (PE needs ~3us of continuous work to ramp)
        warm_done = False

        # persistent activations
        acts = big.enter_context(tc.tile_pool(name="acts", bufs=1))
        qhat = acts.tile([128, 2, T], BF16)    # q^T rotary, heads (2m, 2m+1)
        khat = acts.tile([128, 2, T], BF16)
        vhat = acts.tile([128, NT, HL * 65], BF16)  # v natural + ones col/head
        vhat8 = acts.tile([128, NT, HL * 128], FP8)  # 128/head: v|ones|zeros
        ynhat = acts.tile([128, 2, T], BF16)   # normalized y^T for projection
        # zero-fill vhat8 once (cols 65:128 of each head stay zero forever)
        nc.gpsimd.memset(vhat8, 0.0)

        # working pools (whole-kernel scope)
        xpool = big.enter_context(tc.tile_pool(name="xpool", bufs=2))
        rot = big.enter_context(tc.tile_pool(name="rot", bufs=3))
        ppool = big.enter_context(tc.tile_pool(name="ppool", bufs=3))
        lpool = big.enter_context(tc.tile_pool(name="lpool", bufs=3))
        dpool = big.enter_context(tc.tile_pool(name="dpool", bufs=3, space="DRAM"))
        otpool = big.enter_context(tc.tile_pool(name="ot", bufs=3))

        qkps = big.enter_context(tc.tile_pool(name="qkps", bufs=2, space="PSUM"))
        spool = big.enter_context(tc.tile_pool(name="spool", bufs=2, space="PSUM"))
        ypool = big.enter_context(tc.tile_pool(name="ypool", bufs=1, space="PSUM"))

        # ------------------------------------------------------------------
        # QKV work units for one chunk (list of closures)
        # ------------------------------------------------------------------
        def qkv_units(ch):
            units = []
            fp8c = ch > 0
            sl = slice(ch * 512, (ch + 1) * 512)
            cell = {}
            xdt = FP8 if fp8c else BF16

            def load_x(ch=ch, xdt=xdt, fp8c=fp8c):
                xc = xpool.tile([128, NKT, 512], xdt, tag="xc", name="xc")
                cell['xc'] = xc
                if fp8c:
                    nc.sync.dma_start(out=xc, in_=xT8[:, ch - 1])
                else:
                    for q in range(4):
                        nc.sync.dma_start(
                            out=xc[:, 2 * q:2 * q + 2],
                            in_=xTb[:, 2 * q:2 * q + 2])
            units.append(load_x)

            def drain_qk(ps, qk2, half, fp8c):
                # psum -> joint bf16 tile [128, 2(qk), 512]
                if fp8c:
                    nc.vector.tensor_scalar_mul(qk2[:, half, :], ps, 1.0 / WS)
                else:
                    nc.vector.tensor_copy(qk2[:, half, :], ps)

            def rot_tail(qk2, m, sl):
                # signed rotate-half. ch0/ch1 (critical path to the next
                # attention start): PE permutation matmul, low latency.
                # ch2-3: SBUF-SBUF shift DMAs, latency hidden by the longer
                # preceding attention chunks.
                if ch <= 1:
                    qs2p = [qkps.tile([128, 512], F32, tag="qkps",
                                      name="qs2p") for _ in range(2)]
                    for half in range(2):
                        nc.tensor.matmul(
                            qs2p[half], lhsT=perm_sb, rhs=qk2[:, half, :],
                            start=True, stop=True)
                    shalves = qs2p
                else:
                    qs2 = rot.tile([128, 2, 512], BF16, tag="qs", name="qs")
                    for b0 in (0, 64):
                        nc.sync.dma_start(
                            out=qs2[b0:b0 + 32], in_=qk2[b0 + 32:b0 + 64])
                        nc.sync.dma_start(
                            out=qs2[b0 + 32:b0 + 64], in_=qk2[b0:b0 + 32])
                    shalves = [qs2[:, 0, :], qs2[:, 1, :]]
                for half, dst in ((0, qhat), (1, khat)):
                    u = rot.tile([128, 512], BF16, tag="u", name="u")
                    nc.vector.tensor_mul(u, qk2[:, half, :], cos_sb[:, sl])
                    t_t = rot.tile([128, 512], BF16, tag="t", name="t")
                    nc.vector.tensor_mul(t_t, shalves[half], sin_sb[:, sl])
                    nc.vector.tensor_add(dst[:, m, sl], u, t_t)

            def emit_m(m):
                msl = slice(m * 128, (m + 1) * 128)
                qk2_cell = {}
                mu = []

                def mk_qk2(qk2_cell=qk2_cell):
                    qk2_cell['t'] = rot.tile([128, 2, 512], BF16, tag="qk2",
                                             name="qk2")

                for wi, (wsb, wsb8) in enumerate(
                        ((wq_sb, wq8_sb), (wk_sb, wk8_sb))):
                    if fp8c:
                        def fill_h1(wsb8=wsb8, msl=msl, wi=wi,
                                    qk2_cell=qk2_cell, mk=mk_qk2):
                            if wi == 0:
                                mk()
                            ps = qkps.tile([128, 512], F32, tag="qkps",
                                           name="ps")
                            cell[('ps', wi)] = ps
                            xc = cell['xc']
                            for j in range(2):
                                nc.tensor.matmul(
                                    ps, lhsT=wsb8[:, 2 * j:2 * j + 2, msl],
                                    rhs=xc[:, 2 * j:2 * j + 2, :],
                                    start=(j == 0), stop=False, perf_mode=DR)

                        def fill_h2(wsb8=wsb8, msl=msl, wi=wi,
                                    qk2_cell=qk2_cell):
                            ps = cell[('ps', wi)]
                            xc = cell['xc']
                            for j in range(2, 4):
                                nc.tensor.matmul(
                                    ps, lhsT=wsb8[:, 2 * j:2 * j + 2, msl],
                                    rhs=xc[:, 2 * j:2 * j + 2, :],
                                    start=False, stop=(j == 3), perf_mode=DR)
                            drain_qk(ps, qk2_cell['t'], wi, True)
                        mu.append(fill_h1)
                        mu.append(fill_h2)
                    else:
                        def fill_h1(wsb=wsb, msl=msl, wi=wi,
                                    qk2_cell=qk2_cell, mk=mk_qk2):
                            if wi == 0:
                                mk()
                            ps = qkps.tile([128, 512], F32, tag="qkps",
                                           name="ps")
                            cell[('ps', wi)] = ps
                            xc = cell['xc']
                            for kt in range(4):
                                nc.tensor.matmul(
                                    ps, lhsT=wsb[:, kt, msl], rhs=xc[:, kt, :],
                                    start=(kt == 0), stop=False)

                        def fill_h2(wsb=wsb, msl=msl, wi=wi,
                                    qk2_cell=qk2_cell):
                            ps = cell[('ps', wi)]
                            xc = cell['xc']
                            for kt in range(4, NKT):
                                nc.tensor.matmul(
                                    ps, lhsT=wsb[:, kt, msl], rhs=xc[:, kt, :],
                                    start=False, stop=(kt == NKT - 1))
                            drain_qk(ps, qk2_cell['t'], wi, False)
                        mu.append(fill_h1)
                        mu.append(fill_h2)

                def rotu(m=m, sl=sl, qk2_cell=qk2_cell):
                    rot_tail(qk2_cell['t'], m, sl)
                mu.append(rotu)
                return mu

            def vfill_units():
                vu = []
                for ts in range(4):
                    tt = ch * 4 + ts

                    def vfill(ts=ts, tt=tt, fp8c=fp8c):
                        xc = cell['xc']
                        tsl = slice(ts * 128, (ts + 1) * 128)
                        vp = qkps.tile([128, HL * 65], F32, tag="qkps",
                                       name="vp")
                        if fp8c:
                            for j in range(4):
                                nc.tensor.matmul(
                                    vp, lhsT=xc[:, 2 * j:2 * j + 2, tsl],
                                    rhs=wv8_sb[:, 2 * j:2 * j + 2, :],
                                    start=(j == 0), stop=(j == 3),
                                    perf_mode=DR)
                            nc.vector.tensor_scalar_mul(
                                vhat[:, tt, :], vp, 1.0 / WS)
                        else:
                            for kt in range(NKT):
                                nc.tensor.matmul(
                                    vp, lhsT=xc[:, kt, tsl],
                                    rhs=wv_sb[:, kt, :],
                                    start=(kt == 0), stop=(kt == NKT - 1))
                            nc.vector.tensor_copy(vhat[:, tt, :], vp)
                        for h in range(HL):
                            nc.gpsimd.memset(
                                vhat[:, tt, 65 * h + 64:65 * h + 65], 1.0)
                        # fp8 copy (v + ones cols; zero cols untouched)
                        src = vhat[:, tt, :].rearrange("p (h c) -> p h c", h=HL)
                        dst8 = vhat8[:, tt, :].rearrange(
                            "p (h c) -> p h c", h=HL)[:, :, 0:65]
                        nc.gpsimd.tensor_copy(dst8, src)
                    vu.append(vfill)
                return vu

            # order: x, m0 q/k/rot, v fills, m1 q/k/rot -- so the next
            # chunk's pair-0 attention can start as early as possible
            units.extend(emit_m(0))
            units.extend(vfill_units())
            units.extend(emit_m(1))
            return units

        # ------------------------------------------------------------------
        # proj work units for one chunk
        # ------------------------------------------------------------------
        def proj_units(chp):
            units = []
            for tt in range(4 * chp, 4 * chp + 4):
                def punit(tt=tt):
                    tsl = slice(tt * 128, (tt + 1) * 128)
                    pp = spool.tile([128, 2, 512], F32, tag="s", name="pp")
                    for h2 in range(2):
                        nsl = slice(h2 * 512, (h2 + 1) * 512)
                        for kt in range(2):
                            nc.tensor.matmul(
                                pp[:, h2, :],
                                lhsT=ynhat[:, kt, tsl],
                                rhs=wp_sb[:, kt, nsl],
                                start=(kt == 0), stop=(kt == 1))
                    ot = otpool.tile([128, C], BF16, tag="ot", name="ot")
                    nc.vector.tensor_copy(ot[:, 0:512], pp[:, 0, :])
                    nc.scalar.copy(ot[:, 512:1024], pp[:, 1, :])
                    nc.gpsimd.dma_start(out=out[tsl, :], in_=ot)
                units.append(punit)
            return units

        # ------------------------------------------------------------------
        # attention for one chunk, interleaving pending units
        # ------------------------------------------------------------------
        def emit_attention(ch, pending, reserve=3):
            ic = ch
            i0, i1 = 512 * ic, 512 * (ic + 1)
            jt_hi = min(NT, 4 * (ic + 1))
            full = 4 * ic                      # full blocks; always even
            total_iters = 2 * (full + 4)  # pop points: every jt, both pairs
            n_pend = len(pending)
            reserve = min(3, n_pend)  # keep some PE work for the tail chain
            n_paced = n_pend - reserve
            it = 0
            popped = 0

            def pop_quota():
                nonlocal popped, it
                it += 1
                want = min(n_paced,
                           (n_paced * it + total_iters - 1) // total_iters)
                while popped < want:
                    pending.popleft()()
                    popped += 1

            for pair in range(2):
                ys = [ypool.tile([128, 512], F32, tag=f"y{a}", name=f"y{a}")
                      for a in range(2)]
                ysb = lpool.tile([128, 2, 512], F32, tag="ysb", name="ysb")
                # full blocks: fp8 DoubleRow AV over jt pairs, pipelined
                # at single-jt granularity (sp ring keeps 1-jt lookahead)
                pe8 = None
                for jt in range(full):
                    ji = jt % 2
                    sp = spool.tile([128, 2, 512], F32, tag="s", name="sp")
                    for a in range(2):
                        asl = slice(64 * a, 64 * a + 64)
                        nc.tensor.matmul(
                            sp[:, a, :],
                            lhsT=khat[asl, pair, jt * 128:(jt + 1) * 128],
                            rhs=qhat[asl, pair, i0:i1],
                            start=True, stop=True)
                    if ji == 0:
                        pe8 = ppool.tile([128, 2, 2, 512], FP8, tag="pe8",
                                         name="pe8")
                    if jt == full - 1:  # jt=4ic-1: near block, needs bias
                        pt = ppool.tile([128, 2, 512], BF16, tag="pt",
                                        name="pt")
                        nc.scalar.activation(
                            pt, sp, mybir.ActivationFunctionType.Exp,
                            scale=SCALE)
                        nc.vector.tensor_mul(
                            pe8[:, 1], pt,
                            etab_sb[:, 2 * pair:2 * pair + 2, 128:640])
                    else:
                        nc.scalar.activation(
                            pe8[:, ji], sp,
                            mybir.ActivationFunctionType.Exp, scale=SCALE)
                    if ji == 1:
                        jt0 = jt - 1
                        for a in range(2):
                            h = 2 * pair + a
                            nc.tensor.matmul(
                                ys[a],
                                lhsT=vhat8[:, jt0:jt0 + 2,
                                           128 * h:128 * h + 128],
                                rhs=pe8[:, :, a, :],
                                start=(jt0 == 0), stop=False, perf_mode=DR,
                                skip_group_check=True)
                    pop_quota()
                # diagonal blocks (bf16)
                for jt in range(4 * ic, jt_hi):
                    i_lo = jt * 128
                    n = i1 - i_lo
                    sp = spool.tile([128, 2, 512], F32, tag="s", name="sp")
                    for a in range(2):
                        asl = slice(64 * a, 64 * a + 64)
                        nc.tensor.matmul(
                            sp[:, a, :n],
                            lhsT=khat[asl, pair, jt * 128:(jt + 1) * 128],
                            rhs=qhat[asl, pair, i_lo:i1],
                            start=True, stop=True)
                    pt = ppool.tile([128, 2, 512], BF16, tag="pt", name="pt")
                    nc.scalar.activation(
                        pt[:, :, :n], sp[:, :, :n],
                        mybir.ActivationFunctionType.Exp, scale=SCALE)
                    pe = ppool.tile([128, 2, 512], BF16, tag="pe", name="pe")
                    nc.vector.tensor_mul(
                        pe[:, :, :n], pt[:, :, :n],
                        etab_sb[:, 2 * pair:2 * pair + 2, 0:n])
                    for a in range(2):
                        h = 2 * pair + a
                        nc.tensor.matmul(
                            ys[a][0:65, i_lo - i0:512],
                            lhsT=vhat[:, jt, 65 * h:65 * h + 65],
                            rhs=pe[:, a, :n],
                            start=(ic == 0 and jt == 0),
                            stop=(jt == jt_hi - 1),
                            skip_group_check=True)
                    pop_quota()

                # eagerly drain y psum -> sbuf so the next pair's AV can
                # reuse the psum banks without waiting for the chain below
                nc.vector.tensor_copy(ysb[0:65, 0, :], ys[0][0:65, :])
                nc.vector.tensor_copy(ysb[0:65, 1, :], ys[1][0:65, :])
                # softmax denominators + normalize (batched a=0,1)
                ld = dpool.tile([1, 1024], F32, tag="ld", name="ld")
                nc.sync.dma_start(out=ld, in_=ysb[64:65].rearrange("p a c -> p (a c)"))
                l128 = lpool.tile([128, 8], F32, tag="l128", name="l128")
                nc.sync.dma_start(
                    out=l128, in_=ld.rearrange("a (p c) -> (a p) c", p=128))
                r128 = lpool.tile([128, 8], F32, tag="r128", name="r128")
                nc.vector.reciprocal(r128, l128)
                rd = dpool.tile([1, 1024], F32, tag="rd", name="rd")
                nc.sync.dma_start(
                    out=rd.rearrange("a (p c) -> (a p) c", p=128), in_=r128)
                rb2 = lpool.tile([64, 2, 512], F32, tag="rb2", name="rb2")
                r_bcast = bass.AP(
                    tensor=rd.tensor, offset=rd.offset,
                    ap=[[0, 64]] + list(rd.rearrange(
                        "a (h c) -> a h c", h=2).ap[1:]))
                nc.sync.dma_start(out=rb2, in_=r_bcast)
                for a in range(2):
                    nc.vector.tensor_mul(
                        ynhat[64 * a:64 * a + 64, pair, i0:i1],
                        ysb[0:64, a, :], rb2[:, a, :])

            # drain any leftover units
            while pending:
                pending.popleft()()

        # ------------------------------------------------------------------
        # main schedule
        # ------------------------------------------------------------------
        # PE warmup: garbage matmuls on the memset tile ramp the clock
        # while the const DMAs stream in; also prime the exp act table
        wps = spool.tile([128, 2, 512], F32, tag="s", name="wps")
        for i in range(6):
            nc.tensor.matmul(wps[:, 0, :], lhsT=warm[:, 0:128], rhs=warm,
                             start=(i == 0), stop=(i == 5))
        warmx = consts.tile([1, 8], BF16)
        nc.scalar.activation(warmx, warm[0:1, 0:8],
                             mybir.ActivationFunctionType.Exp)

        u0 = qkv_units(0)
        # emit x + m0 + v fills now; m1 units become att(0) filler
        for u in u0[:10]:
            u()
        pend0 = deque(u0[10:])
        pend0.extend(qkv_units(1))
        emit_attention(0, pend0)
        pend1 = deque(qkv_units(2))
        pend1.extend(proj_units(0))
        emit_attention(1, pend1)
        emit_attention(2, deque(qkv_units(3)))
        pend3 = deque(proj_units(1))
        pend3.extend(proj_units(2))
        emit_attention(3, pend3, reserve=3)
        for u in proj_units(NCH - 1):
            u()

    return nc


# ---------------------------------------------------------------------------
# Host-side input preparation
# ---------------------------------------------------------------------------

def _rotary_tables():
    inv_freq = (1.0 / (ROTARY_BASE ** (
        np.arange(0, D, 2, dtype=np.float32) / D))).astype(np.float32)
    t = np.arange(T, dtype=np.float32)
    freqs = np.einsum('i,j->ij', t, inv_freq).astype(np.float32)  # [T, 32]
    freqs = np.concatenate([freqs, freqs], axis=1)                # [T, 64]
    cos = np.cos(freqs).T.astype(np.float32)                      # [64, T]
    sin = np.sin(freqs).T.astype(np.float32)
    # stack for two heads per 128-partition tile
    cosT = np.concatenate([cos, cos], axis=0)                     # [128, T]
    sinN = np.concatenate([sin, sin], axis=0).copy()
    # shifted-term coefficient indexed by DEST row (the shifted copy is
    # materialized before the multiply): rows 0:32 get -sin, 32:64 get +sin
    sinN[0:32] *= -1.0
    sinN[64:96] *= -1.0
    return (np.ascontiguousarray(cosT).astype(BF16_NP),
            np.ascontiguousarray(sinN).astype(BF16_NP))


def _perm_matrix():
    """Plain rotate-half permutation (signs live in sinN): swap 32-row
    halves within each 64-row head block."""
    P = np.zeros((128, 128), dtype=np.float32)
    for b in (0, 64):
        for j in range(32):
            P[b + 32 + j, b + j] = 1.0
            P[b + j, b + 32 + j] = 1.0
    return P.astype(BF16_NP)


def _bucket(d):
    """T5 causal relative-position bucket for distance d = i - j >= 0."""
    d = np.asarray(d)
    max_exact = NUM_BUCKETS // 2
    is_small = d < max_exact
    dsafe = np.maximum(d, 1).astype(np.float32)
    val = max_exact + (
        np.log(dsafe / max_exact) / math.log(MAX_DISTANCE / max_exact)
        * (NUM_BUCKETS - max_exact)
    ).astype(np.int32)
    val = np.minimum(val, NUM_BUCKETS - 1)
    return np.where(is_small, d, val)


def _etab_for_heads(rel_bias_table, heads):
    """exp((bias-b31)/sqrt(D)) block-Toeplitz table [len(heads), 128, ETW].
    Column k*128+ii, row jj -> distance 128k + ii - jj; negative -> 0 (mask).
    The per-head bucket-31 bias is subtracted (softmax shift-invariance), so
    any block at distance offset >= 256 is exactly 1.0 and skips the lookup.
    """
    ii = np.arange(128)
    jj = np.arange(128)
    out = np.zeros((len(heads), 128, ETW), dtype=np.float32)
    dmax = ETW
    dist_all = np.arange(0, dmax)
    buck = _bucket(dist_all)  # [ETW]
    g = {}
    for hi, h in enumerate(heads):
        b31 = rel_bias_table[NUM_BUCKETS - 1, h].astype(np.float32)
        gh = np.exp((rel_bias_table[buck, h].astype(np.float32) - b31) * SCALE)
        g[h] = gh
    for k in range(ETW // 128):
        dmat = 128 * k + ii[None, :] - jj[:, None]  # [jj, ii]
        valid = dmat >= 0
        dcl = np.clip(dmat, 0, dmax - 1)
        for hi, h in enumerate(heads):
            blk = np.where(valid, g[h][dcl], 0.0)
            out[hi, :, 128 * k:128 * (k + 1)] = blk
    return out.astype(BF16_NP)


_NC_CACHE = None


def _prearr(w):
    """[K, N] -> [128, K//128, N] partition-contiguous layout."""
    k, n = w.shape
    return np.ascontiguousarray(w.reshape(k // 128, 128, n).transpose(1, 0, 2))


def _pad_wv(wv_slice):
    """[C, 256] -> [C, 260]: per head 64 cols + a zero col (ones col target)."""
    out = np.zeros((C, HL * 65), dtype=np.float32)
    for h in range(HL):
        out[:, 65 * h:65 * h + 64] = wv_slice[:, 64 * h:64 * h + 64]
    return out


def _build_in_maps(inputs):
    x = np.asarray(inputs["x"], dtype=np.float32)
    Wq = np.asarray(inputs["Wq"], dtype=np.float32)
    Wk = np.asarray(inputs["Wk"], dtype=np.float32)
    Wv = np.asarray(inputs["Wv"], dtype=np.float32)
    Wp = np.asarray(inputs["Wp"], dtype=np.float32)
    rel_bias_table = np.asarray(inputs["rel_bias_table"], dtype=np.float32)

    cosT, sinN = _rotary_tables()
    in_maps = []
    for core in range(N_CORES):
        b = core // 4
        hg = core % 4
        heads = list(range(4 * hg, 4 * hg + 4))
        csl = slice(DHL * hg, DHL * (hg + 1))
        xt = x[b].T.astype(np.float32)           # [C, T]
        xr = np.ascontiguousarray(
            xt.reshape(NKT, 128, NCH, 512).transpose(1, 2, 0, 3))
        wvp = _pad_wv(Wv[:, csl])
        in_maps.append({
            "xTb": xr[:, 0].astype(BF16_NP),
            "xT8": xr[:, 1:].astype(FP8_NP),
            "wq": _prearr(Wq[:, csl]).astype(BF16_NP),
            "wk": _prearr(Wk[:, csl]).astype(BF16_NP),
            "wv": _prearr(wvp).astype(BF16_NP),
            "wq8": _prearr(Wq[:, csl] * WS).astype(FP8_NP),
            "wk8": _prearr(Wk[:, csl] * WS).astype(FP8_NP),
            "wv8": _prearr(wvp * WS).astype(FP8_NP),
            "wp": _prearr(Wp[csl, :]).astype(BF16_NP),
            "cosT": cosT,
            "sinN": sinN,
            "perm": _perm_matrix(),
            "etab": _etab_for_heads(rel_bias_table, heads).reshape(
                2, 2, 128, ETW).transpose(0, 2, 1, 3).copy(),
        })
    return in_maps


def kernel(x, Wq, bq, Wk, bk, Wv, bv, Wp, bp, rel_bias_table):
    global _NC_CACHE
    if _NC_CACHE is None:
        _NC_CACHE = build_nc()
    nc = _NC_CACHE

    in_maps = _build_in_maps({
        "x": x, "Wq": Wq, "Wk": Wk, "Wv": Wv, "Wp": Wp,
        "rel_bias_table": rel_bias_table,
    })

    res = run_bass_kernel_spmd(nc, in_maps, list(range(N_CORES)))

    out = np.zeros((B, T, C), dtype=np.float32)
    for core in range(N_CORES):
        out[core // 4] += np.asarray(res.results[core]["out"], dtype=np.float32)
    out += np.asarray(bp, dtype=np.float32)[None, None, :]
    return out


# revision 69
# speedup vs baseline: 1.0261x; 1.0123x over previous
"""Trainium2 Bass kernel for causal self-attention with rotary + T5-style
relative-position bias (nn_CausalSelfAttention_27195732918417).

Sharding: 8 cores = 2 batches x 4 head-groups (4 heads each).
Each core computes its 4 heads end-to-end and a partial output projection;
the host sums the 4 partials per batch.

v3 design notes:
- softmax shift-invariance: the T5 bucket saturates at bucket 31 for all
  distances >= 113, so subtracting the per-head bucket-31 bias from the
  whole table leaves a bias of exactly 0 for all "far" blocks (block
  offset >= 256).  Far blocks need no bias multiply; the Toeplitz
  exp-bias table only needs 640 columns.
- fp8 (e4m3) DoubleRow matmuls: QKV projections for chunks 1-3 (weights
  host-scaled x32 to avoid fp8 subnormals, undone in the psum drain) and
  the attention AV for all full (non-diagonal) blocks.  V is padded to
  128 columns per head (64 v + ones col + 63 zeros) so the softmax
  denominator rides in the same DR matmul (stream-bound: free).
  Chunk 0 stays bf16: short-prefix rows average over few positions and
  need the precision.  exp() writes fp8 directly for far blocks.
- interleaved emission: the PE instruction stream alternates attention
  (ch), QKV (ch+1) and proj (ch-1) work so no engine starves.
- output in bf16 (host accumulates partials in f32).

Self-contained: hardcodes B=2, T=2048, C=1024, H=16, D=64.
"""

import math
import sys
import types
from collections import deque

import numpy as np
import ml_dtypes

# ---------------------------------------------------------------------------
# Environment patches (axon agent container)
# ---------------------------------------------------------------------------


def _install_ntff_hook():
    """Provide antenv.axon_hooks (missing in this image) so trace=True works."""
    try:
        from antenv.axon_hooks import get_axon_ntff_profile_hook  # noqa: F401
        return
    except ImportError:
        pass
    try:
        from trn_agent_boot.trn_boot import _ntff_profile_via_ctypes
        hook = _ntff_profile_via_ctypes('/opt/axon/libaxon_pjrt.so')
    except Exception:
        hook = None
    mod = types.ModuleType('antenv.axon_hooks')
    mod.get_axon_ntff_profile_hook = lambda: hook
    mod.set_axon_ntff_profile_hook = lambda h: None
    sys.modules['antenv.axon_hooks'] = mod


def _patch_tile_drain():
    """This container's walrus rejects >1 sync-wait per instruction.

    Two patches:
    1. tail drain: split its waits across multiple drain instructions
    2. general: before lowering, split any instruction with >1 waits by
       inserting standalone InstEventSemaphore carriers before it on the
       same engine (engine streams execute in order, so happens-before is
       preserved).
    """
    import concourse.mybir as mybir
    import concourse.tile as tile
    from concourse.tile import ScopedClock

    def _drain_and_barrier_split(self, tick_clock, wait_clock):
        nc = self.nc
        drain_inst = nc.sync.drain()
        wait_clock.add_sem_waits(
            drain_inst.ins, ScopedClock({None: tick_clock.global_clock})
        )
        si = drain_inst.ins.sync_info
        waits = list(si.on_wait) if si and si.on_wait else []
        if len(waits) > 1:
            si.on_wait = waits[:1]
            for w in waits[1:]:
                extra = nc.sync.drain()
                esi = extra.ins.sync_info
                if esi is None:
                    extra.ins.sync_info = mybir.SyncInfo(on_wait=[w], on_update=[])
                else:
                    esi.on_wait = [w]

        nc.all_engine_barrier()
        assert self.sems is not None
        popped = nc._tile_sem_poison_stack.pop()
        assert popped is self._sem_poison
        nc.clear_and_free_semaphores(list(self.sems.allocated().values()))
        nc.all_engine_barrier()

    tile.TileContext._drain_and_barrier = _drain_and_barrier_split

    orig_lower = tile.TileContext._lower_ordered_insts

    def _lower_split_waits(self, ordered):
        nc = self.nc
        for bb_name, insts in ordered.items():
            new_insts = []
            for inst in insts:
                si = getattr(inst, "sync_info", None)
                waits = list(si.on_wait) if si and si.on_wait else []
                if len(waits) > 1 and inst.engine != mybir.EngineType.Unassigned:
                    for w in waits[:-1]:
                        carrier = mybir.InstEventSemaphore(
                            name=nc.get_next_instruction_name(),
                            engine=inst.engine,
                            ins=[],
                            outs=[],
                            sync_info=mybir.SyncInfo(on_wait=[w], on_update=[]),
                        )
                        new_insts.append(carrier)
                    si.on_wait = waits[-1:]
                new_insts.append(inst)
            insts[:] = new_insts
        return orig_lower(self, ordered)

    tile.TileContext._lower_ordered_insts = _lower_split_waits


_install_ntff_hook()
_patch_tile_drain()

import concourse.bass as bass  # noqa: E402
import concourse.mybir as mybir  # noqa: E402
import concourse.tile as tile  # noqa: E402
from concourse.bass_utils import run_bass_kernel_spmd  # noqa: E402

# ---------------------------------------------------------------------------
# Problem constants
# ---------------------------------------------------------------------------
B, T, C = 2, 2048, 1024
H = 16            # total heads
D = 64            # head dim
HL = 4            # heads per core
DHL = HL * D      # 256 local channels
N_CORES = 8
NUM_BUCKETS = 32
MAX_DISTANCE = 128
ROTARY_BASE = 10000.0
SCALE = 1.0 / math.sqrt(D)
WS = 32.0         # fp8 weight pre-scale (avoids e4m3 subnormals)

F32 = mybir.dt.float32
BF16 = mybir.dt.bfloat16
FP8 = mybir.dt.float8e4
BF16_NP = ml_dtypes.bfloat16
FP8_NP = ml_dtypes.float8_e4m3
DR = mybir.MatmulPerfMode.DoubleRow

NT = T // 128     # 16 t-tiles
NKT = C // 128    # 8 contraction tiles
NCH = T // 512    # 4 streaming chunks
ETW = 640         # exp-bias table width (distances < 640 after b31 shift)


# ---------------------------------------------------------------------------
# Device program (identical on all cores; data differs)
# ---------------------------------------------------------------------------

def build_nc():
    from contextlib import ExitStack

    nc = bass.Bass()

    xTb = nc.dram_tensor("xTb", [128, NKT, 512], BF16, kind="ExternalInput")
    xT8 = nc.dram_tensor("xT8", [128, 3, NKT, 512], FP8, kind="ExternalInput")
    wq = nc.dram_tensor("wq", [128, NKT, DHL], BF16, kind="ExternalInput")
    wk = nc.dram_tensor("wk", [128, NKT, DHL], BF16, kind="ExternalInput")
    wv = nc.dram_tensor("wv", [128, NKT, HL * 65], BF16, kind="ExternalInput")
    wq8 = nc.dram_tensor("wq8", [128, NKT, DHL], FP8, kind="ExternalInput")
    wk8 = nc.dram_tensor("wk8", [128, NKT, DHL], FP8, kind="ExternalInput")
    wv8 = nc.dram_tensor("wv8", [128, NKT, HL * 65], FP8, kind="ExternalInput")
    wp = nc.dram_tensor("wp", [128, 2, C], BF16, kind="ExternalInput")
    cosT = nc.dram_tensor("cosT", [128, T], BF16, kind="ExternalInput")
    sinN = nc.dram_tensor("sinN", [128, T], BF16, kind="ExternalInput")
    # exp((bias - b31)/sqrt(D)) Toeplitz blocks, [pair][128][2 heads][ETW]
    etab = nc.dram_tensor("etab", [2, 128, 2, ETW], BF16, kind="ExternalInput")
    # signed rotate-half permutation matrix (out = perm.T @ in)
    perm = nc.dram_tensor("perm", [128, 128], BF16, kind="ExternalInput")
    out = nc.dram_tensor("out", [T, C], BF16, kind="ExternalOutput")

    with tile.TileContext(nc) as tc, ExitStack() as big:
        consts = big.enter_context(tc.tile_pool(name="consts", bufs=1))

        # const loads: ordered so the first chunk's deps arrive first;
        # split across the two DMA rings (gpsimd + sync issue ~1us each).
        warm = consts.tile([128, 512], BF16)
        nc.gpsimd.memset(warm, 0.0)
        wq_sb = consts.tile([128, NKT, DHL], BF16)
        nc.gpsimd.dma_start(out=wq_sb, in_=wq[:])
        wk_sb = consts.tile([128, NKT, DHL], BF16)
        nc.gpsimd.dma_start(out=wk_sb, in_=wk[:])
        cos_sb = consts.tile([128, T], BF16)
        nc.gpsimd.dma_start(out=cos_sb, in_=cosT[:])
        sin_sb = consts.tile([128, T], BF16)
        nc.gpsimd.dma_start(out=sin_sb, in_=sinN[:])
        perm_sb = consts.tile([128, 128], BF16)
        nc.gpsimd.dma_start(out=perm_sb, in_=perm[:])
        wv_sb = consts.tile([128, NKT, HL * 65], BF16)
        nc.gpsimd.dma_start(out=wv_sb, in_=wv[:])
        wq8_sb = consts.tile([128, NKT, DHL], FP8)
        nc.gpsimd.dma_start(out=wq8_sb, in_=wq8[:])
        wk8_sb = consts.tile([128, NKT, DHL], FP8)
        nc.gpsimd.dma_start(out=wk8_sb, in_=wk8[:])
        wv8_sb = consts.tile([128, NKT, HL * 65], FP8)
        nc.gpsimd.dma_start(out=wv8_sb, in_=wv8[:])
        etab_sb = consts.tile([128, HL, ETW], BF16)
        for pr in range(2):
            nc.gpsimd.dma_start(out=etab_sb[:, 2 * pr:2 * pr + 2, :], in_=etab[pr])
        wp_sb = consts.tile([128, 2, C], BF16)
        nc.gpsimd.dma_start(out=wp_sb, in_=wp[:])

        # PE p-state warmup: a few garbage matmuls so the first real fills
        # run at full clock (PE needs ~3us of continuous work to ramp)
        warm_done = False

        # persistent activations
        acts = big.enter_context(tc.tile_pool(name="acts", bufs=1))
        qhat = acts.tile([128, 2, T], BF16)    # q^T rotary, heads (2m, 2m+1)
        khat = acts.tile([128, 2, T], BF16)
        vhat = acts.tile([128, NT, HL * 65], BF16)  # v natural + ones col/head
        vhat8 = acts.tile([128, NT, HL * 128], FP8)  # 128/head: v|ones|zeros
        ynhat = acts.tile([128, 2, T], BF16)   # normalized y^T for projection
        # zero-fill vhat8 once (cols 65:128 of each head stay zero forever)
        nc.gpsimd.memset(vhat8, 0.0)

        # working pools (whole-kernel scope)
        xpool = big.enter_context(tc.tile_pool(name="xpool", bufs=2))
        rot = big.enter_context(tc.tile_pool(name="rot", bufs=3))
        ppool = big.enter_context(tc.tile_pool(name="ppool", bufs=3))
        lpool = big.enter_context(tc.tile_pool(name="lpool", bufs=3))
        dpool = big.enter_context(tc.tile_pool(name="dpool", bufs=3, space="DRAM"))
        otpool = big.enter_context(tc.tile_pool(name="ot", bufs=3))

        qkps = big.enter_context(tc.tile_pool(name="qkps", bufs=2, space="PSUM"))
        spool = big.enter_context(tc.tile_pool(name="spool", bufs=2, space="PSUM"))
        ypool = big.enter_context(tc.tile_pool(name="ypool", bufs=1, space="PSUM"))

        # ------------------------------------------------------------------
        # QKV work units for one chunk (list of closures)
        # ------------------------------------------------------------------
        def qkv_units(ch):
            units = []
            fp8c = ch > 0
            sl = slice(ch * 512, (ch + 1) * 512)
            cell = {}
            xdt = FP8 if fp8c else BF16

            def load_x(ch=ch, xdt=xdt, fp8c=fp8c):
                xc = xpool.tile([128, NKT, 512], xdt, tag="xc", name="xc")
                cell['xc'] = xc
                if fp8c:
                    nc.sync.dma_start(out=xc, in_=xT8[:, ch - 1])
                else:
                    for q in range(4):
                        nc.sync.dma_start(
                            out=xc[:, 2 * q:2 * q + 2],
                            in_=xTb[:, 2 * q:2 * q + 2])
            units.append(load_x)

            def drain_qk(ps, qk2, half, fp8c):
                # psum -> joint bf16 tile [128, 2(qk), 512]
                if fp8c:
                    nc.vector.tensor_scalar_mul(qk2[:, half, :], ps, 1.0 / WS)
                else:
                    nc.vector.tensor_copy(qk2[:, half, :], ps)

            def rot_tail(qk2, m, sl):
                # signed rotate-half. ch0/ch1 (critical path to the next
                # attention start): PE permutation matmul, low latency.
                # ch2-3: SBUF-SBUF shift DMAs, latency hidden by the longer
                # preceding attention chunks.
                if ch <= 1:
                    qs2p = [qkps.tile([128, 512], F32, tag="qkps",
                                      name="qs2p") for _ in range(2)]
                    for half in range(2):
                        nc.tensor.matmul(
                            qs2p[half], lhsT=perm_sb, rhs=qk2[:, half, :],
                            start=True, stop=True)
                    shalves = qs2p
                else:
                    qs2 = rot.tile([128, 2, 512], BF16, tag="qs", name="qs")
                    for b0 in (0, 64):
                        nc.sync.dma_start(
                            out=qs2[b0:b0 + 32], in_=qk2[b0 + 32:b0 + 64])
                        nc.sync.dma_start(
                            out=qs2[b0 + 32:b0 + 64], in_=qk2[b0:b0 + 32])
                    shalves = [qs2[:, 0, :], qs2[:, 1, :]]
                for half, dst in ((0, qhat), (1, khat)):
                    u = rot.tile([128, 512], BF16, tag="u", name="u")
                    nc.vector.tensor_mul(u, qk2[:, half, :], cos_sb[:, sl])
                    t_t = rot.tile([128, 512], BF16, tag="t", name="t")
                    nc.vector.tensor_mul(t_t, shalves[half], sin_sb[:, sl])
                    nc.vector.tensor_add(dst[:, m, sl], u, t_t)

            def emit_m(m):
                msl = slice(m * 128, (m + 1) * 128)
                qk2_cell = {}
                mu = []

                def mk_qk2(qk2_cell=qk2_cell):
                    qk2_cell['t'] = rot.tile([128, 2, 512], BF16, tag="qk2",
                                             name="qk2")

                for wi, (wsb, wsb8) in enumerate(
                        ((wq_sb, wq8_sb), (wk_sb, wk8_sb))):
                    if fp8c:
                        def fill_h1(wsb8=wsb8, msl=msl, wi=wi,
                                    qk2_cell=qk2_cell, mk=mk_qk2):
                            if wi == 0:
                                mk()
                            ps = qkps.tile([128, 512], F32, tag="qkps",
                                           name="ps")
                            cell[('ps', wi)] = ps
                            xc = cell['xc']
                            for j in range(2):
                                nc.tensor.matmul(
                                    ps, lhsT=wsb8[:, 2 * j:2 * j + 2, msl],
                                    rhs=xc[:, 2 * j:2 * j + 2, :],
                                    start=(j == 0), stop=False, perf_mode=DR)

                        def fill_h2(wsb8=wsb8, msl=msl, wi=wi,
                                    qk2_cell=qk2_cell):
                            ps = cell[('ps', wi)]
                            xc = cell['xc']
                            for j in range(2, 4):
                                nc.tensor.matmul(
                                    ps, lhsT=wsb8[:, 2 * j:2 * j + 2, msl],
                                    rhs=xc[:, 2 * j:2 * j + 2, :],
                                    start=False, stop=(j == 3), perf_mode=DR)
                            drain_qk(ps, qk2_cell['t'], wi, True)
                        mu.append(fill_h1)
                        mu.append(fill_h2)
                    else:
                        def fill_h1(wsb=wsb, msl=msl, wi=wi,
                                    qk2_cell=qk2_cell, mk=mk_qk2):
                            if wi == 0:
                                mk()
                            ps = qkps.tile([128, 512], F32, tag="qkps",
                                           name="ps")
                            cell[('ps', wi)] = ps
                            xc = cell['xc']
                            for kt in range(4):
                                nc.tensor.matmul(
                                    ps, lhsT=wsb[:, kt, msl], rhs=xc[:, kt, :],
                                    start=(kt == 0), stop=False)

                        def fill_h2(wsb=wsb, msl=msl, wi=wi,
                                    qk2_cell=qk2_cell):
                            ps = cell[('ps', wi)]
                            xc = cell['xc']
                            for kt in range(4, NKT):
                                nc.tensor.matmul(
                                    ps, lhsT=wsb[:, kt, msl], rhs=xc[:, kt, :],
                                    start=False, stop=(kt == NKT - 1))
                            drain_qk(ps, qk2_cell['t'], wi, False)
                        mu.append(fill_h1)
                        mu.append(fill_h2)

                def rotu(m=m, sl=sl, qk2_cell=qk2_cell):
                    rot_tail(qk2_cell['t'], m, sl)
                mu.append(rotu)
                return mu

            def vfill_units():
                vu = []
                for ts in range(4):
                    tt = ch * 4 + ts

                    def vfill(ts=ts, tt=tt, fp8c=fp8c):
                        xc = cell['xc']
                        tsl = slice(ts * 128, (ts + 1) * 128)
                        vp = qkps.tile([128, HL * 65], F32, tag="qkps",
                                       name="vp")
                        if fp8c:
                            for j in range(4):
                                nc.tensor.matmul(
                                    vp, lhsT=xc[:, 2 * j:2 * j + 2, tsl],
                                    rhs=wv8_sb[:, 2 * j:2 * j + 2, :],
                                    start=(j == 0), stop=(j == 3),
                                    perf_mode=DR)
                            nc.vector.tensor_scalar_mul(
                                vhat[:, tt, :], vp, 1.0 / WS)
                        else:
                            for kt in range(NKT):
                                nc.tensor.matmul(
                                    vp, lhsT=xc[:, kt, tsl],
                                    rhs=wv_sb[:, kt, :],
                                    start=(kt == 0), stop=(kt == NKT - 1))
                            nc.vector.tensor_copy(vhat[:, tt, :], vp)
                        for h in range(HL):
                            nc.gpsimd.memset(
                                vhat[:, tt, 65 * h + 64:65 * h + 65], 1.0)
                        # fp8 copy (v + ones cols; zero cols untouched)
                        src = vhat[:, tt, :].rearrange("p (h c) -> p h c", h=HL)
                        dst8 = vhat8[:, tt, :].rearrange(
                            "p (h c) -> p h c", h=HL)[:, :, 0:65]
                        nc.gpsimd.tensor_copy(dst8, src)
                    vu.append(vfill)
                return vu

            # order: x, m0 q/k/rot, v fills, m1 q/k/rot -- so the next
            # chunk's pair-0 attention can start as early as possible
            units.extend(emit_m(0))
            units.extend(vfill_units())
            units.extend(emit_m(1))
            return units

        # ------------------------------------------------------------------
        # proj work units for one chunk
        # ------------------------------------------------------------------
        def proj_units(chp):
            units = []
            for tt in range(4 * chp, 4 * chp + 4):
                def punit(tt=tt):
                    tsl = slice(tt * 128, (tt + 1) * 128)
                    pp = spool.tile([128, 2, 512], F32, tag="s", name="pp")
                    for h2 in range(2):
                        nsl = slice(h2 * 512, (h2 + 1) * 512)
                        for kt in range(2):
                            nc.tensor.matmul(
                                pp[:, h2, :],
                                lhsT=ynhat[:, kt, tsl],
                                rhs=wp_sb[:, kt, nsl],
                                start=(kt == 0), stop=(kt == 1))
                    ot = otpool.tile([128, C], BF16, tag="ot", name="ot")
                    nc.vector.tensor_copy(ot[:, 0:512], pp[:, 0, :])
                    nc.scalar.copy(ot[:, 512:1024], pp[:, 1, :])
                    nc.gpsimd.dma_start(out=out[tsl, :], in_=ot)
                units.append(punit)
            return units

        # ------------------------------------------------------------------
        # attention for one chunk, interleaving pending units
        # ------------------------------------------------------------------
        def emit_attention(ch, pending, reserve=3):
            ic = ch
            i0, i1 = 512 * ic, 512 * (ic + 1)
            jt_hi = min(NT, 4 * (ic + 1))
            full = 4 * ic                      # full blocks; always even
            total_iters = 2 * (full + 4)  # pop points: every jt, both pairs
            n_pend = len(pending)
            reserve = min(3, n_pend)  # keep some PE work for the tail chain
            n_paced = n_pend - reserve
            it = 0
            popped = 0

            def pop_quota():
                nonlocal popped, it
                it += 1
                want = min(n_paced,
                           (n_paced * it + total_iters - 1) // total_iters)
                while popped < want:
                    pending.popleft()()
                    popped += 1

            for pair in range(2):
                ys = [ypool.tile([128, 512], F32, tag=f"y{a}", name=f"y{a}")
                      for a in range(2)]
                ysb = lpool.tile([128, 2, 512], F32, tag="ysb", name="ysb")
                # full blocks: fp8 DoubleRow AV over jt pairs, pipelined
                # at single-jt granularity (sp ring keeps 1-jt lookahead)
                pe8 = None
                for jt in range(full):
                    ji = jt % 2
                    sp = spool.tile([128, 2, 512], F32, tag="s", name="sp")
                    for a in range(2):
                        asl = slice(64 * a, 64 * a + 64)
                        nc.tensor.matmul(
                            sp[:, a, :],
                            lhsT=khat[asl, pair, jt * 128:(jt + 1) * 128],
                            rhs=qhat[asl, pair, i0:i1],
                            start=True, stop=True)
                    if ji == 0:
                        pe8 = ppool.tile([128, 2, 2, 512], FP8, tag="pe8",
                                         name="pe8")
                    if jt == full - 1:  # jt=4ic-1: near block, needs bias
                        pt = ppool.tile([128, 2, 512], BF16, tag="pt",
                                        name="pt")
                        nc.scalar.activation(
                            pt, sp, mybir.ActivationFunctionType.Exp,
                            scale=SCALE)
                        nc.vector.tensor_mul(
                            pe8[:, 1], pt,
                            etab_sb[:, 2 * pair:2 * pair + 2, 128:640])
                    else:
                        nc.scalar.activation(
                            pe8[:, ji], sp,
                            mybir.ActivationFunctionType.Exp, scale=SCALE)
                    if ji == 1:
                        jt0 = jt - 1
                        for a in range(2):
                            h = 2 * pair + a
                            nc.tensor.matmul(
                                ys[a],
                                lhsT=vhat8[:, jt0:jt0 + 2,
                                           128 * h:128 * h + 128],
                                rhs=pe8[:, :, a, :],
                                start=(jt0 == 0), stop=False, perf_mode=DR,
                                skip_group_check=True)
                    pop_quota()
                # diagonal blocks (bf16)
                for jt in range(4 * ic, jt_hi):
                    i_lo = jt * 128
                    n = i1 - i_lo
                    sp = spool.tile([128, 2, 512], F32, tag="s", name="sp")
                    for a in range(2):
                        asl = slice(64 * a, 64 * a + 64)
                        nc.tensor.matmul(
                            sp[:, a, :n],
                            lhsT=khat[asl, pair, jt * 128:(jt + 1) * 128],
                            rhs=qhat[asl, pair, i_lo:i1],
                            start=True, stop=True)
                    pt = ppool.tile([128, 2, 512], BF16, tag="pt", name="pt")
                    nc.scalar.activation(
                        pt[:, :, :n], sp[:, :, :n],
                        mybir.ActivationFunctionType.Exp, scale=SCALE)
                    pe = ppool.tile([128, 2, 512], BF16, tag="pe", name="pe")
                    nc.vector.tensor_mul(
                        pe[:, :, :n], pt[:, :, :n],
                        etab_sb[:, 2 * pair:2 * pair + 2, 0:n])
                    for a in range(2):
                        h = 2 * pair + a
                        nc.tensor.matmul(
                            ys[a][0:65, i_lo - i0:512],
                            lhsT=vhat[:, jt, 65 * h:65 * h + 65],
                            rhs=pe[:, a, :n],
                            start=(ic == 0 and jt == 0),
                            stop=(jt == jt_hi - 1),
                            skip_group_check=True)
                    pop_quota()

                # eagerly drain y psum -> sbuf so the next pair's AV can
                # reuse the psum banks without waiting for the chain below.
                # split across vector+scalar: they run in parallel, and the
                # act engine is idle at pair boundaries anyway
                nc.vector.tensor_copy(ysb[0:65, 0, :], ys[0][0:65, :])
                nc.scalar.copy(ysb[0:65, 1, :], ys[1][0:65, :])
                # softmax denominators + normalize (batched a=0,1)
                ld = dpool.tile([1, 1024], F32, tag="ld", name="ld")
                nc.sync.dma_start(out=ld, in_=ysb[64:65].rearrange("p a c -> p (a c)"))
                l128 = lpool.tile([128, 8], F32, tag="l128", name="l128")
                nc.sync.dma_start(
                    out=l128, in_=ld.rearrange("a (p c) -> (a p) c", p=128))
                r128 = lpool.tile([128, 8], F32, tag="r128", name="r128")
                nc.vector.reciprocal(r128, l128)
                rd = dpool.tile([1, 1024], F32, tag="rd", name="rd")
                nc.sync.dma_start(
                    out=rd.rearrange("a (p c) -> (a p) c", p=128), in_=r128)
                rb2 = lpool.tile([64, 2, 512], F32, tag="rb2", name="rb2")
                r_bcast = bass.AP(
                    tensor=rd.tensor, offset=rd.offset,
                    ap=[[0, 64]] + list(rd.rearrange(
                        "a (h c) -> a h c", h=2).ap[1:]))
                nc.sync.dma_start(out=rb2, in_=r_bcast)
                for a in range(2):
                    nc.vector.tensor_mul(
                        ynhat[64 * a:64 * a + 64, pair, i0:i1],
                        ysb[0:64, a, :], rb2[:, a, :])

            # drain any leftover units
            while pending:
                pending.popleft()()

        # ------------------------------------------------------------------
        # main schedule
        # ------------------------------------------------------------------
        # PE warmup: garbage matmuls on the memset tile ramp the clock
        # while the const DMAs stream in; also prime the exp act table
        wps = spool.tile([128, 2, 512], F32, tag="s", name="wps")
        for i in range(6):
            nc.tensor.matmul(wps[:, 0, :], lhsT=warm[:, 0:128], rhs=warm,
                             start=(i == 0), stop=(i == 5))
        warmx = consts.tile([1, 8], BF16)
        nc.scalar.activation(warmx, warm[0:1, 0:8],
                             mybir.ActivationFunctionType.Exp)

        u0 = qkv_units(0)
        # emit x + m0 + v fills now; m1 units become att(0) filler
        for u in u0[:10]:
            u()
        pend0 = deque(u0[10:])
        pend0.extend(qkv_units(1))
        emit_attention(0, pend0)
        pend1 = deque(qkv_units(2))
        pend1.extend(proj_units(0))
        emit_attention(1, pend1)
        emit_attention(2, deque(qkv_units(3)))
        pend3 = deque(proj_units(1))
        pend3.extend(proj_units(2))
        emit_attention(3, pend3, reserve=3)
        for u in proj_units(NCH - 1):
            u()

    return nc


# ---------------------------------------------------------------------------
# Host-side input preparation
# ---------------------------------------------------------------------------

def _rotary_tables():
    inv_freq = (1.0 / (ROTARY_BASE ** (
        np.arange(0, D, 2, dtype=np.float32) / D))).astype(np.float32)
    t = np.arange(T, dtype=np.float32)
    freqs = np.einsum('i,j->ij', t, inv_freq).astype(np.float32)  # [T, 32]
    freqs = np.concatenate([freqs, freqs], axis=1)                # [T, 64]
    cos = np.cos(freqs).T.astype(np.float32)                      # [64, T]
    sin = np.sin(freqs).T.astype(np.float32)
    # stack for two heads per 128-partition tile
    cosT = np.concatenate([cos, cos], axis=0)                     # [128, T]
    sinN = np.concatenate([sin, sin], axis=0).copy()
    # shifted-term coefficient indexed by DEST row (the shifted copy is
    # materialized before the multiply): rows 0:32 get -sin, 32:64 get +sin
    sinN[0:32] *= -1.0
    sinN[64:96] *= -1.0
    return (np.ascontiguousarray(cosT).astype(BF16_NP),
            np.ascontiguousarray(sinN).astype(BF16_NP))


def _perm_matrix():
    """Plain rotate-half permutation (signs live in sinN): swap 32-row
    halves within each 64-row head block."""
    P = np.zeros((128, 128), dtype=np.float32)
    for b in (0, 64):
        for j in range(32):
            P[b + 32 + j, b + j] = 1.0
            P[b + j, b + 32 + j] = 1.0
    return P.astype(BF16_NP)


def _bucket(d):
    """T5 causal relative-position bucket for distance d = i - j >= 0."""
    d = np.asarray(d)
    max_exact = NUM_BUCKETS // 2
    is_small = d < max_exact
    dsafe = np.maximum(d, 1).astype(np.float32)
    val = max_exact + (
        np.log(dsafe / max_exact) / math.log(MAX_DISTANCE / max_exact)
        * (NUM_BUCKETS - max_exact)
    ).astype(np.int32)
    val = np.minimum(val, NUM_BUCKETS - 1)
    return np.where(is_small, d, val)


def _etab_for_heads(rel_bias_table, heads):
    """exp((bias-b31)/sqrt(D)) block-Toeplitz table [len(heads), 128, ETW].
    Column k*128+ii, row jj -> distance 128k + ii - jj; negative -> 0 (mask).
    The per-head bucket-31 bias is subtracted (softmax shift-invariance), so
    any block at distance offset >= 256 is exactly 1.0 and skips the lookup.
    """
    ii = np.arange(128)
    jj = np.arange(128)
    out = np.zeros((len(heads), 128, ETW), dtype=np.float32)
    dmax = ETW
    dist_all = np.arange(0, dmax)
    buck = _bucket(dist_all)  # [ETW]
    g = {}
    for hi, h in enumerate(heads):
        b31 = rel_bias_table[NUM_BUCKETS - 1, h].astype(np.float32)
        gh = np.exp((rel_bias_table[buck, h].astype(np.float32) - b31) * SCALE)
        g[h] = gh
    for k in range(ETW // 128):
        dmat = 128 * k + ii[None, :] - jj[:, None]  # [jj, ii]
        valid = dmat >= 0
        dcl = np.clip(dmat, 0, dmax - 1)
        for hi, h in enumerate(heads):
            blk = np.where(valid, g[h][dcl], 0.0)
            out[hi, :, 128 * k:128 * (k + 1)] = blk
    return out.astype(BF16_NP)


_NC_CACHE = None


def _prearr(w):
    """[K, N] -> [128, K//128, N] partition-contiguous layout."""
    k, n = w.shape
    return np.ascontiguousarray(w.reshape(k // 128, 128, n).transpose(1, 0, 2))


def _pad_wv(wv_slice):
    """[C, 256] -> [C, 260]: per head 64 cols + a zero col (ones col target)."""
    out = np.zeros((C, HL * 65), dtype=np.float32)
    for h in range(HL):
        out[:, 65 * h:65 * h + 64] = wv_slice[:, 64 * h:64 * h + 64]
    return out


def _build_in_maps(inputs):
    x = np.asarray(inputs["x"], dtype=np.float32)
    Wq = np.asarray(inputs["Wq"], dtype=np.float32)
    Wk = np.asarray(inputs["Wk"], dtype=np.float32)
    Wv = np.asarray(inputs["Wv"], dtype=np.float32)
    Wp = np.asarray(inputs["Wp"], dtype=np.float32)
    rel_bias_table = np.asarray(inputs["rel_bias_table"], dtype=np.float32)

    cosT, sinN = _rotary_tables()
    in_maps = []
    for core in range(N_CORES):
        b = core // 4
        hg = core % 4
        heads = list(range(4 * hg, 4 * hg + 4))
        csl = slice(DHL * hg, DHL * (hg + 1))
        xt = x[b].T.astype(np.float32)           # [C, T]
        xr = np.ascontiguousarray(
            xt.reshape(NKT, 128, NCH, 512).transpose(1, 2, 0, 3))
        wvp = _pad_wv(Wv[:, csl])
        in_maps.append({
            "xTb": xr[:, 0].astype(BF16_NP),
            "xT8": xr[:, 1:].astype(FP8_NP),
            "wq": _prearr(Wq[:, csl]).astype(BF16_NP),
            "wk": _prearr(Wk[:, csl]).astype(BF16_NP),
            "wv": _prearr(wvp).astype(BF16_NP),
            "wq8": _prearr(Wq[:, csl] * WS).astype(FP8_NP),
            "wk8": _prearr(Wk[:, csl] * WS).astype(FP8_NP),
            "wv8": _prearr(wvp * WS).astype(FP8_NP),
            "wp": _prearr(Wp[csl, :]).astype(BF16_NP),
            "cosT": cosT,
            "sinN": sinN,
            "perm": _perm_matrix(),
            "etab": _etab_for_heads(rel_bias_table, heads).reshape(
                2, 2, 128, ETW).transpose(0, 2, 1, 3).copy(),
        })
    return in_maps


def kernel(x, Wq, bq, Wk, bk, Wv, bv, Wp, bp, rel_bias_table):
    global _NC_CACHE
    if _NC_CACHE is None:
        _NC_CACHE = build_nc()
    nc = _NC_CACHE

    in_maps = _build_in_maps({
        "x": x, "Wq": Wq, "Wk": Wk, "Wv": Wv, "Wp": Wp,
        "rel_bias_table": rel_bias_table,
    })

    res = run_bass_kernel_spmd(nc, in_maps, list(range(N_CORES)))

    out = np.zeros((B, T, C), dtype=np.float32)
    for core in range(N_CORES):
        out[core // 4] += np.asarray(res.results[core]["out"], dtype=np.float32)
    out += np.asarray(bp, dtype=np.float32)[None, None, :]
    return out
